# revision 30
# baseline (speedup 1.0000x reference)
"""CrossGSA fused attention kernel for 8x Trainium2 NeuronCores.

Sharding: each core owns one batch (b = core//4) and a 576-query-token slice
(qs = core%4) of that batch, across ALL 8 heads.  k/v are computed full per
core (per-batch); the mask is the dominant traffic and is read once per
batch across the machine (each core reads mask[:, :, its q-slice],
pre-transposed on host to bf16 so DMA lines stay contiguous).

Device layouts are transposed ([channel, token]) end-to-end:
  - q/k/v/o projections run in bf16 (weights and activations converted on
    host; the f32 residual path keeps a separate f32 x-slice),
  - projections and rotary are pipelined per 512-column chunk so attention
    starts as soon as the first key tiles are rotated; rotary runs in bf16
    on the DVE (pair-swap via SBUF-SBUF DMA),
  - S^T accumulates over the head dim (K=32, 4-way tile-position packed) on
    top of the mask, which is injected into PSUM by an identity-matmul
    (the identity matmul also provides the full-row start of the PSUM
    accumulation group: a row-positioned start=True wedges the PE),
  - exp() runs on the scalar engine straight out of PSUM,
  - attn@v uses v in natural [token, dim] layout as the stationary operand,
    extended per head with a ones column so the softmax denominator falls
    out of the same matmul (no separate denominator matmuls); two heads
    pack per PSUM tile at tile_position cols {0, 64}, channels in rows
    0-31/64-95, denominators in rows 32/96; a DMA pass remaps rows to the
    channel layout afterwards,
  - the attention mt-loop is software-pipelined with a 2-tile skew
    (exp/attn@v of tile mt issue after S of tile mt+2) and the first two
    mask tiles are prefetched ahead of the bulk x DMAs,
  - the depthwise 5x5 lepe conv runs in bf16 on the DVE, overlapped with
    attention (lepe is only needed at the output projection),
  - layernorm stats use ones-matmuls; mean/rstd broadcast back via K=1
    matmuls.
The per-core [256, 576] transposed outputs are gathered and untransposed on
host.

run_with_iters() executes through a cached jax.jit callable with
device-resident inputs, so repeated calls measure device execution only
(the executable is loaded once, not per call).
"""

import numpy as np
import ml_dtypes

import concourse.bass as bass
import concourse.mybir as mybir
import concourse.tile as tile
from concourse import bacc, bass_utils

F32 = mybir.dt.float32
F32R = mybir.dt.float32r
BF16 = mybir.dt.bfloat16
F8 = mybir.dt.float8e4
AF = mybir.ActivationFunctionType
ALU = mybir.AluOpType

B, H, W, C = 2, 48, 48, 256
NH, HD = 8, 32
N = H * W            # 2304 tokens per batch
NQ = N // 4          # 576 query tokens per core
NCORES = 8
SCALING = HD ** -0.5
LN_EPS = 1e-6
MT = N // 128        # 18 key tiles
ROWS_Q = NQ // W     # 12 image rows per core
HALO = ROWS_Q + 4    # rows incl. conv halo
NHT = HALO * W       # 768 halo tokens
QCH = [(0, 512), (512, 64)]                                  # q chunks (bank)
NCH = [(0, 512), (512, 512), (1024, 512), (1536, 512), (2048, 256)]
HCH = [(0, 512), (512, 256)]

_PROGS = {}
_RUNNERS = {}
ABLATE = set()


def _bcast_ap(src, n=128):
    return bass.AP(tensor=src.tensor, offset=src.offset,
                   ap=[[0, n]] + src.ap[1:])


def _build_program(iters=1):
    nc = bacc.Bacc("TRN2", target_bir_lowering=False, debug=False,
                   enable_asserts=False, num_devices=NCORES)

    def din(name, shape, dt=F32):
        return nc.dram_tensor(name, shape, dt, kind="ExternalInput").ap()

    io = dict(
        xT=din("xT", [C, N], BF16),
        xTq=din("xTq", [C, NQ]),
        xTq_bf=din("xTq_bf", [C, NQ], BF16),
        xTh=din("xTh", [C, NHT], BF16),
        valid=din("valid", [1, NHT]),
        maskT=din("maskT", [NH, N, NQ], BF16),     # mask, transposed
        sinT=din("sinT", [HD, N], BF16),
        cosT=din("cosT", [HD, N], BF16),
        wq=din("wq", [C, C], BF16), wk=din("wk", [C, C], BF16),
        wv=din("wv", [C, C], BF16), wo=din("wo", [C, C], BF16),
        bq=din("bq", [C, 1]), bk=din("bk", [C, 1]),
        bv=din("bv", [C, 1]), bo=din("bo", [C, 1]),
        bv_row=din("bv_row", [1, C]),
        lepe_w=din("lepe_w", [C, 25]), lepe_b=din("lepe_b", [C, 1]),
        ln_g=din("ln_g", [C, 1]), ln_b=din("ln_b", [C, 1]),
        ones_p=din("ones_p", [128, 1]),
        ident_bf=din("ident_bf", [128, 128], BF16),
        ones_k1=din("ones_k1", [1, 128]),
        blockind=din("blockind", [4, 128]),
        outT=nc.dram_tensor("outT", [C, NQ], F32, kind="ExternalOutput").ap(),
    )
    with tile.TileContext(nc) as tc:
        with tc.tile_pool(name="persist", bufs=1) as P:
            for it in range(iters):
                _emit(nc, tc, P, io, it)
    nc.compile()
    return nc


def _emit(nc, tc, P, io, it=0):
    dma = nc.sync.dma_start

    def pt(tg, shape, dt=F32):
        return P.tile(shape, dt, tag=tg, name=f"{tg}_{it}")

    # ---- constants / inputs to SBUF ----
    x_sb = [pt(f"x_sb{k}", [128, N], BF16) for k in range(2)]
    xq_sb = [pt(f"xq_sb{k}", [128, NQ]) for k in range(2)]
    xqb_sb = [pt(f"xqb_sb{k}", [128, NQ], BF16) for k in range(2)]
    xh_sb = [pt(f"xh_sb{k}", [128, NHT], BF16) for k in range(2)]
    w_sb, b_sb = {}, {}
    for nm in ("wq", "wk", "wv", "wo"):
        t = pt(f"{nm}_sb", [128, 2, C], BF16)
        for kk in range(2):
            dma(t[:, kk, :], io[nm][128 * kk:128 * (kk + 1), :])
        w_sb[nm] = t
    for nm in ("bq", "bk", "bv", "bo", "lepe_b", "ln_g", "ln_b"):
        t = pt(f"{nm}_sb", [128, 2, 1])
        for kk in range(2):
            dma(t[:, kk, :], io[nm][128 * kk:128 * (kk + 1), :])
        b_sb[nm] = t
    lw_sb = pt("lw_sb", [128, 2, 25])
    for kk in range(2):
        dma(lw_sb[:, kk, :], io["lepe_w"][128 * kk:128 * (kk + 1), :])
    for k in range(2):
        for off, wd in QCH:
            dma(xqb_sb[k][:, off:off + wd],
                io["xTq_bf"][128 * k:128 * (k + 1), off:off + wd])
        dma(xq_sb[k][:], io["xTq"][128 * k:128 * (k + 1), :])
        for off, wd in NCH:
            dma(x_sb[k][:, off:off + wd],
                io["xT"][128 * k:128 * (k + 1), off:off + wd])
        dma(xh_sb[k][:], io["xTh"][128 * k:128 * (k + 1), :])
    o1_sb = pt("o1_sb", [128, 1]); dma(o1_sb[:], io["ones_p"][:])
    id_sb = pt("id_sb", [128, 128], BF16); dma(id_sb[:], io["ident_bf"][:])
    ok1_sb = pt("ok1_sb", [1, 128]); dma(ok1_sb[:], io["ones_k1"][:])
    bi_sb = pt("bi_sb", [4, 128]); dma(bi_sb[:], io["blockind"][:])
    # prefetch the first two mask tiles of group 0 ahead of the big x DMAs
    # so the attention pipeline has mask data as soon as s/exp are ready
    msk_pre = [pt(f"msk_pre{i}", [128, 4, NQ], BF16) for i in range(4)]
    mio = io["maskT"]
    for i in range(4):
        if "maskdma" not in ABLATE:
            dma(msk_pre[i][:], bass.AP(
                tensor=mio.tensor, offset=mio.offset + 128 * i * NQ,
                ap=[[NQ, 128], [N * NQ, 4], [1, NQ]]))
        else:
            nc.vector.memset(msk_pre[i][:], 1.0)
    bvr_sb = pt("bvr_sb", [128, C]); dma(bvr_sb[:], _bcast_ap(io["bv_row"]))
    val_sb = pt("val_sb", [128, NHT]); dma(val_sb[:], _bcast_ap(io["valid"]))
    sin_sb = pt("sin_sb", [128, N], BF16)
    cos_sb = pt("cos_sb", [128, N], BF16)
    dma(sin_sb[:], bass.AP(tensor=io["sinT"].tensor, offset=io["sinT"].offset,
                           ap=[[0, 4]] + io["sinT"].ap))
    dma(cos_sb[:], bass.AP(tensor=io["cosT"].tensor, offset=io["cosT"].offset,
                           ap=[[0, 4]] + io["cosT"].ap))

    kr_bf = [pt(f"kr_bf{k}", [128, N], BF16) for k in range(2)]
    qr_bf = [pt(f"qr_bf{k}", [128, NQ], BF16) for k in range(2)]
    # v extended per head: [32 ch | ones | zeros] -> 64-wide stationary blocks
    vn_bf = pt("vn_bf", [128, MT, NH, 33], BF16)
    vh_sb = [pt(f"vh_sb{k}", [128, NHT], BF16) for k in range(2)]
    vpad = [pt(f"vpad{k}", [128, HALO, W + 4], BF16) for k in range(2)]
    lepe_sb = [pt(f"lepe_sb{k}", [128, NQ], BF16) for k in range(2)]
    oat_sb = [pt(f"oat_sb{k}", [128, NQ]) for k in range(2)]
    of_sb = [pt(f"of_sb{k}", [128, NQ], BF16) for k in range(2)]
    o2_sb = [pt(f"o2_sb{k}", [128, NQ]) for k in range(2)]
    sq_sb = [pt(f"sq_sb{k}", [128, NQ]) for k in range(2)]
    ofull = [pt(f"ofull{k}", [128, 2, NQ]) for k in range(2)]
    rden4 = [pt(f"rden4{k}", [4, NQ]) for k in range(2)]
    rb_sb = pt("rb_sb", [128, NQ])
    m1_sb = pt("m1_sb", [1, NQ])
    msq_sb = pt("msq_sb", [1, NQ])
    var_sb = pt("var_sb", [1, NQ])
    rstd_sb = pt("rstd_sb", [1, NQ])
    eps_sb = pt("eps_sb", [1, 1])
    nc.vector.memset(eps_sb[:], LN_EPS)
    outf_sb = [pt(f"outf_sb{k}", [128, NQ]) for k in range(2)]

    if it == 0:
        # ones column of the v blocks; untouched by later writes, so
        # steady-state iterations skip the init.
        nc.vector.memset(vn_bf[:, :, :, 32:33], 1.0)

    # ---- Phase 1: projections + rotary, pipelined per 512-col chunk so
    # attention can start as soon as the first key tiles are rotated ----
    kTb = [pt(f"kTb{k}", [128, N], BF16) for k in range(2)]
    qTb = [pt(f"qTb{k}", [128, NQ], BF16) for k in range(2)]

    def rotary_chunk(pjs, srcT, dst, off, wd):
        # out = x*cos + pairswap(x)*sin_signed, all bf16 (2x DVE rate);
        # pair-swap partitions via SBUF->SBUF DMA (engines cannot
        # read/write strided partitions)
        shuf = pjs.tile([128, 512], BF16, tag="shuf", name="shuf")
        t1 = pjs.tile([128, 512], BF16, tag="t1", name="t1")
        dma(shuf[0:128:2, :wd], srcT[1:128:2, off:off + wd])
        dma(shuf[1:128:2, :wd], srcT[0:128:2, off:off + wd])
        nc.vector.tensor_tensor(t1[:, :wd], srcT[:, off:off + wd],
                                cos_sb[:, off:off + wd], op=ALU.mult)
        nc.vector.tensor_tensor(shuf[:, :wd], shuf[:, :wd],
                                sin_sb[:, off:off + wd], op=ALU.mult)
        nc.vector.tensor_tensor(dst[:, off:off + wd], t1[:, :wd],
                                shuf[:, :wd], op=ALU.add)

    with tc.tile_pool(name=f"pj_{it}", bufs=3, space="PSUM") as pj, \
         tc.tile_pool(name=f"pjs_{it}", bufs=4) as pjs:
        def emit_q(jt, off, wd):
            ps = pj.tile([128, 512], F32, tag="ps", name="ps")
            for kk in range(2):
                nc.tensor.matmul(ps[:, :wd],
                                 w_sb["wq"][:, kk, 128 * jt:128 * (jt + 1)],
                                 xqb_sb[kk][:, off:off + wd],
                                 start=(kk == 0), stop=(kk == 1))
            nc.vector.tensor_scalar_add(qTb[jt][:, off:off + wd],
                                        ps[:, :wd], b_sb["bq"][:, jt, :])

        def emit_k(jt, off, wd):
            ps = pj.tile([128, 512], F32, tag="ps", name="ps")
            for kk in range(2):
                nc.tensor.matmul(ps[:, :wd],
                                 w_sb["wk"][:, kk, 128 * jt:128 * (jt + 1)],
                                 x_sb[kk][:, off:off + wd],
                                 start=(kk == 0), stop=(kk == 1))
            nc.vector.tensor_scalar(kTb[jt][:, off:off + wd], ps[:, :wd],
                                    b_sb["bk"][:, jt, :], SCALING,
                                    op0=ALU.add, op1=ALU.mult)

        # v first: av(mt) only needs vn(mt), keep it off the critical chain
        for mt in range(MT):
            ps = pj.tile([128, 256], F32, tag="psv", name="psv")
            for kk in range(2):
                nc.tensor.matmul(ps[:],
                                 x_sb[kk][:, 128 * mt:128 * (mt + 1)],
                                 w_sb["wv"][:, kk, :],
                                 start=(kk == 0), stop=(kk == 1))
            nc.vector.tensor_tensor(
                vn_bf[:, mt, :, 0:32],
                ps[:].rearrange("p (h c) -> p h c", c=HD),
                bvr_sb[:].rearrange("p (h c) -> p h c", c=HD), op=ALU.add)
        # All projection matmuls + PSUM evacuations first (the evacs are the
        # last PSUM readers, so the projection banks free early for phase 2);
        # the rotary chains read only SBUF and trail behind, critical
        # (g0: kT chunk 0 + full qT) chunks first.
        for jt in range(2):
            emit_k(jt, *NCH[0])
            for off, wd in QCH:
                emit_q(jt, off, wd)
            for off, wd in NCH[1:]:
                emit_k(jt, off, wd)
        for jt in range(2):
            for off, wd in HCH:
                ps = pj.tile([128, 512], F32, tag="ps", name="ps")
                for kk in range(2):
                    nc.tensor.matmul(ps[:, :wd],
                                     w_sb["wv"][:, kk, 128 * jt:128 * (jt + 1)],
                                     xh_sb[kk][:, off:off + wd],
                                     start=(kk == 0), stop=(kk == 1))
                # vT_halo = valid*bv + psum (keeps zero-padding exact)
                nc.vector.scalar_tensor_tensor(vh_sb[jt][:, off:off + wd],
                                               val_sb[:, off:off + wd],
                                               b_sb["bv"][:, jt, :],
                                               ps[:, :wd],
                                               op0=ALU.mult, op1=ALU.add)
        if "rotary" not in ABLATE:
            for jt in range(2):
                rotary_chunk(pjs, kTb[jt], kr_bf[jt], *NCH[0])
                for off, wd in QCH:
                    rotary_chunk(pjs, qTb[jt], qr_bf[jt], off, wd)
                for off, wd in NCH[1:]:
                    rotary_chunk(pjs, kTb[jt], kr_bf[jt], off, wd)

    if "rotary" in ABLATE:
        for jt in range(2):
            nc.vector.memset(kr_bf[jt][:], 0.01)
            nc.vector.memset(qr_bf[jt][:], 0.01)

    # ---- Phase 1b: depthwise 5x5 lepe conv on GPSIMD ----
    if "conv" in ABLATE:
        for jt in range(2):
            nc.vector.memset(lepe_sb[jt][:], 0.0)
    for jt in range(2) if "conv" not in ABLATE else []:
        nc.gpsimd.memset(vpad[jt][:], 0.0)
        nc.gpsimd.tensor_copy(
            vpad[jt][:, :, 2:2 + W],
            vh_sb[jt][:].rearrange("p (r w) -> p r w", w=W))
        lp = lepe_sb[jt][:].rearrange("p (r w) -> p r w", w=W)
        first = True
        for dy in range(5):
            for dx in range(5):
                src = vpad[jt][:, dy:dy + ROWS_Q, dx:dx + W]
                wtap = lw_sb[:, jt, 5 * dy + dx:5 * dy + dx + 1]
                if first:
                    nc.vector.tensor_scalar(lp, src, wtap,
                                            b_sb["lepe_b"][:, jt, :],
                                            op0=ALU.mult, op1=ALU.add)
                    first = False
                else:
                    nc.vector.scalar_tensor_tensor(lp, src, wtap, lp,
                                                   op0=ALU.mult, op1=ALU.add)

    # ---- Phase 2: attention, two 4-head supergroups ----
    for g in range(2):
        with tc.tile_pool(name=f"op{g}_{it}", bufs=1, space="PSUM") as op:
            o_t = [op.tile([128, 512], F32, tag=f"o_t{pr}", name=f"o_t{pr}")
                   for pr in range(2)]
            o_s = op.tile([128, 2, 64], F32, tag="o_s", name="o_s")
            with tc.tile_pool(name=f"sp{g}_{it}", bufs=2, space="PSUM") as sp, \
                 tc.tile_pool(name=f"stp{g}_{it}", bufs=1, space="PSUM") as stp, \
                 tc.tile_pool(name=f"mp{g}_{it}", bufs=3) as mp, \
                 tc.tile_pool(name=f"pp{g}_{it}", bufs=3) as pp:
                live = {}

                def emit_s(mt):
                    if g == 0 and mt < 4:
                        msk = msk_pre[mt]
                    else:
                        msk = mp.tile([128, 4, NQ], BF16, tag="msk",
                                      name="msk")
                        mio = io["maskT"]
                        src = bass.AP(
                            tensor=mio.tensor,
                            offset=mio.offset + (4 * g) * N * NQ
                            + 128 * mt * NQ,
                            ap=[[NQ, 128], [N * NQ, 4], [1, NQ]])
                        if "maskdma" not in ABLATE:
                            dma(msk[:], src)
                        else:
                            nc.vector.memset(msk[:], 1.0)
                    s_pair = [sp.tile([128, 2, 512], F32, tag="s", name="s")
                              for _ in range(2)]
                    stub = stp.tile([128, 4, 64], F32, tag="stub", name="stub")
                    for j in range(4) if "smm" not in ABLATE else []:
                        pr, ln_ = j // 2, j % 2
                        lhs = kr_bf[g][32 * j:32 * (j + 1),
                                       128 * mt:128 * (mt + 1)]
                        rq = qr_bf[g]
                        # inject mask via identity matmul: provides the
                        # full-row start of the PSUM group (a row-positioned
                        # start=True wedges the PE on this hardware)
                        nc.tensor.matmul(s_pair[pr][:, ln_, :], id_sb[:],
                                         msk[:, j, 0:512],
                                         start=True, stop=False)
                        nc.tensor.matmul(s_pair[pr][:, ln_, :], lhs,
                                         rq[32 * j:32 * (j + 1), 0:512],
                                         start=False, stop=True,
                                         tile_position=(32 * j, 0))
                        nc.tensor.matmul(stub[:, j, :], id_sb[:],
                                         msk[:, j, 512:576],
                                         start=True, stop=False)
                        nc.tensor.matmul(stub[:, j, :], lhs,
                                         rq[32 * j:32 * (j + 1), 512:576],
                                         start=False, stop=True,
                                         tile_position=(32 * j, 0))
                    if "smm" in ABLATE:
                        for pr in range(2):
                            nc.vector.memset(s_pair[pr][:], 0.01)
                        nc.vector.memset(stub[:], 0.01)
                    live[mt] = (msk, s_pair, stub)

                def emit_epa(mt):
                    # exp + mask-multiply + attn@v for tile mt; issued after
                    # s(mt+1) so the PE stream never stalls on the Act/DVE
                    # chain of the current tile.
                    msk, s_pair, stub = live.pop(mt)
                    p_sb = pp.tile([128, 4, NQ], BF16, tag="p_sb", name="p_sb")
                    if "exp" in ABLATE:
                        nc.vector.memset(p_sb[:], 0.001)
                    if "exp" not in ABLATE:
                        for pr in range(2):
                            nc.scalar.activation(p_sb[:, 2 * pr:2 * pr + 2, 0:512],
                                                 s_pair[pr][:], AF.Exp)
                        nc.scalar.activation(p_sb[:, :, 512:576], stub[:], AF.Exp)
                    for j in range(4) if "av" not in ABLATE else []:
                        pr, ln_ = j // 2, j % 2
                        h = 4 * g + j
                        lhsv = vn_bf[:, mt, h, :]
                        nc.tensor.matmul(o_t[pr][64 * ln_:64 * ln_ + 33, :],
                                         lhsv, p_sb[:, j, 0:512],
                                         start=(mt == 0), stop=(mt == MT - 1),
                                         tile_position=(0, 64 * ln_))
                        nc.tensor.matmul(o_s[64 * ln_:64 * ln_ + 33, pr, :],
                                         lhsv, p_sb[:, j, 512:576],
                                         start=(mt == 0), stop=(mt == MT - 1),
                                         tile_position=(0, 64 * ln_))

                if "noskew" in ABLATE:
                    for mt in range(MT):
                        emit_s(mt)
                        emit_epa(mt)
                else:
                    for mt in range(MT):
                        emit_s(mt)
                        if mt > 1:
                            emit_epa(mt - 2)
                    emit_epa(MT - 2)
                    emit_epa(MT - 1)
            if "av" in ABLATE:
                nc.vector.memset(oat_sb[g][:], 0.01)
                nc.vector.tensor_tensor(of_sb[g][:], oat_sb[g][:],
                                        lepe_sb[g][:], op=ALU.add)
                continue
            # evacuate lane-preserving, then DMA-remap rows:
            # head j=2*pr+ln at psum rows 64*ln..64*ln+32 of tile pr,
            # denominator at row 64*ln+32.
            for pr in range(2):
                nc.vector.tensor_copy(ofull[g][:, pr, 0:512], o_t[pr][:])
                nc.vector.tensor_copy(ofull[g][:, pr, 512:576], o_s[:, pr, :])
            for pr in range(2):
                dma(rden4[g][2 * pr:2 * pr + 2, :],
                    ofull[g][32:128:64, pr, :])
            for j in range(4):
                pr, ln_ = j // 2, j % 2
                dma(oat_sb[g][32 * j:32 * (j + 1), :],
                    ofull[g][64 * ln_:64 * ln_ + 32, pr, :])
            nc.vector.reciprocal(rden4[g][:], rden4[g][:])
            with tc.tile_pool(name=f"rb{g}_{it}", bufs=1, space="PSUM") as rbp:
                rb_m = rbp.tile([128, 512], F32, tag="rb_m", name="rb_m")
                rb_s = rbp.tile([128, 64], F32, tag="rb_s", name="rb_s")
                nc.tensor.matmul(rb_m[:], bi_sb[:], rden4[g][:, 0:512],
                                 start=True, stop=True)
                nc.tensor.matmul(rb_s[:], bi_sb[:], rden4[g][:, 512:576],
                                 start=True, stop=True)
                nc.vector.tensor_copy(rb_sb[:, 0:512], rb_m[:])
                nc.vector.tensor_copy(rb_sb[:, 512:576], rb_s[:])
            nc.vector.tensor_tensor(oat_sb[g][:], oat_sb[g][:], rb_sb[:],
                                    op=ALU.mult)
        nc.vector.tensor_tensor(of_sb[g][:], oat_sb[g][:], lepe_sb[g][:],
                                op=ALU.add)

    # ---- Phase 3: out-projection + residual + layernorm ----
    with tc.tile_pool(name=f"pwp_{it}", bufs=2, space="PSUM") as pwp:
        for jt in range(2):
            for off, wd in QCH:
                ps = pwp.tile([128, 512], F32, tag="pw", name="pw")
                for kk in range(2):
                    nc.tensor.matmul(ps[:, :wd],
                                     w_sb["wo"][:, kk, 128 * jt:128 * (jt + 1)],
                                     of_sb[kk][:, off:off + wd],
                                     start=(kk == 0), stop=(kk == 1))
                nc.vector.scalar_tensor_tensor(o2_sb[jt][:, off:off + wd],
                                               ps[:, :wd],
                                               b_sb["bo"][:, jt, :],
                                               xq_sb[jt][:, off:off + wd],
                                               op0=ALU.add, op1=ALU.add)
            nc.vector.tensor_tensor(sq_sb[jt][:], o2_sb[jt][:], o2_sb[jt][:],
                                    op=ALU.mult)
    with tc.tile_pool(name=f"stat_{it}", bufs=1, space="PSUM") as st:
        mu, ssq = {}, {}
        for off, wd in QCH:
            mu[off] = st.tile([1, wd], F32, tag=f"mu{off}", name="mu")
            ssq[off] = st.tile([1, wd], F32, tag=f"ssq{off}", name="ssq")
            for jt in range(2):
                nc.tensor.matmul(mu[off][:], o1_sb[:],
                                 o2_sb[jt][:, off:off + wd],
                                 start=(jt == 0), stop=(jt == 1))
                nc.tensor.matmul(ssq[off][:], o1_sb[:],
                                 sq_sb[jt][:, off:off + wd],
                                 start=(jt == 0), stop=(jt == 1))
        for off, wd in QCH:
            sl = slice(off, off + wd)
            nc.vector.tensor_scalar_mul(m1_sb[:, sl], mu[off][:], 1.0 / C)
            nc.vector.tensor_tensor(msq_sb[:, sl], m1_sb[:, sl],
                                    m1_sb[:, sl], op=ALU.mult)
            nc.vector.scalar_tensor_tensor(var_sb[:, sl], ssq[off][:],
                                           1.0 / C, msq_sb[:, sl],
                                           op0=ALU.mult, op1=ALU.subtract)
        nc.scalar.activation(rstd_sb[:], var_sb[:], AF.Sqrt, bias=eps_sb[:])
        nc.vector.reciprocal(rstd_sb[:], rstd_sb[:])
    with tc.tile_pool(name=f"bc_{it}", bufs=1, space="PSUM") as bc:
        mb = bc.tile([128, 512], F32, tag="mb", name="mb")
        mbs = bc.tile([128, 64], F32, tag="mbs", name="mbs")
        rbm = bc.tile([128, 512], F32, tag="rbm", name="rbm")
        rbs = bc.tile([128, 64], F32, tag="rbs", name="rbs")
        nc.tensor.matmul(mb[:], ok1_sb[:], m1_sb[:, 0:512],
                         start=True, stop=True)
        nc.tensor.matmul(mbs[:], ok1_sb[:], m1_sb[:, 512:576],
                         start=True, stop=True)
        nc.tensor.matmul(rbm[:], ok1_sb[:], rstd_sb[:, 0:512],
                         start=True, stop=True)
        nc.tensor.matmul(rbs[:], ok1_sb[:], rstd_sb[:, 512:576],
                         start=True, stop=True)
        mb_sb = sq_sb[0]  # scratch reuse
        rs_sb = rb_sb
        nc.vector.tensor_copy(mb_sb[:, 0:512], mb[:])
        nc.vector.tensor_copy(mb_sb[:, 512:576], mbs[:])
        nc.vector.tensor_copy(rs_sb[:, 0:512], rbm[:])
        nc.vector.tensor_copy(rs_sb[:, 512:576], rbs[:])
    for jt in range(2):
        t1 = oat_sb[jt]  # scratch reuse
        nc.vector.tensor_tensor(t1[:], o2_sb[jt][:], mb_sb[:],
                                op=ALU.subtract)
        nc.vector.tensor_tensor(t1[:], t1[:], rs_sb[:], op=ALU.mult)
        nc.vector.affine_then_add(outf_sb[jt][:], t1[:], o2_sb[jt][:],
                                  b_sb["ln_g"][:, jt, :],
                                  b_sb["ln_b"][:, jt, :])
        dma(io["outT"][128 * jt:128 * (jt + 1), :], outf_sb[jt][:])


def _host_inputs(x, sin, cos, mask, wq, bq, wk, bk, wv, bv,
                 lepe_w, lepe_b, wo, bo, ln_g, ln_b):
    bf = ml_dtypes.bfloat16
    maskT = np.ascontiguousarray(
        np.transpose(np.asarray(mask, np.float32), (0, 2, 1))).astype(bf)
    pm1 = np.tile(np.array([-1.0, 1.0], np.float32), HD // 2).reshape(HD, 1)
    sinT = np.ascontiguousarray(
        (np.asarray(sin, np.float32).reshape(N, HD).T * pm1).astype(bf))
    cosT = np.ascontiguousarray(
        np.asarray(cos, np.float32).reshape(N, HD).T.astype(bf))
    col = lambda a: np.asarray(a, np.float32).reshape(C, 1)
    common = dict(
        sinT=sinT, cosT=cosT,
        wq=np.asarray(wq, np.float32).astype(bf),
        wk=np.asarray(wk, np.float32).astype(bf),
        wv=np.asarray(wv, np.float32).astype(bf),
        wo=np.asarray(wo, np.float32).astype(bf),
        bq=col(bq), bk=col(bk), bv=col(bv), bo=col(bo),
        bv_row=np.asarray(bv, np.float32).reshape(1, C),
        lepe_w=np.asarray(lepe_w, np.float32).reshape(C, 25),
        lepe_b=col(lepe_b), ln_g=col(ln_g), ln_b=col(ln_b),
        ones_p=np.ones((128, 1), np.float32),
        ident_bf=np.eye(128, dtype=bf),
        ones_k1=np.ones((1, 128), np.float32),
        blockind=np.repeat(np.eye(4, dtype=np.float32), 32, axis=1
                           ).reshape(4, 128),
    )
    xs = np.asarray(x, np.float32).reshape(B, N, C)
    in_maps = []
    for c in range(NCORES):
        bb, qs = c // 4, c % 4
        xTb = np.ascontiguousarray(xs[bb].T)
        q0 = qs * NQ
        r0 = qs * ROWS_Q - 2
        halo = np.zeros((C, NHT), bf)
        vmask = np.zeros((1, NHT), np.float32)
        for r in range(HALO):
            ri = r0 + r
            if 0 <= ri < H:
                halo[:, r * W:(r + 1) * W] = xTb[:, ri * W:(ri + 1) * W].astype(bf)
                vmask[0, r * W:(r + 1) * W] = 1.0
        xq_f = np.ascontiguousarray(xTb[:, q0:q0 + NQ])
        m = dict(common)
        m.update(xT=xTb.astype(bf), xTq=xq_f, xTq_bf=xq_f.astype(bf),
                 xTh=halo, valid=vmask,
                 maskT=np.ascontiguousarray(maskT[:, :, q0:q0 + NQ]))
        in_maps.append(m)
    return in_maps


def _get_prog(iters=1):
    key = (iters, tuple(sorted(ABLATE)))
    if key not in _PROGS:
        _PROGS[key] = _build_program(iters)
    return _PROGS[key]


class _CachedRunner:
    """jax.jit callable built once per program; inputs device-put once per
    in_maps object, so repeated calls measure device execution only."""

    def __init__(self, nc, n_cores=NCORES):
        import jax
        from jax.sharding import Mesh, PartitionSpec
        from jax.experimental.shard_map import shard_map
        from concourse import bass2jax

        bass2jax.install_neuronx_cc_hook()
        self._jax = jax
        part = nc.partition_id_tensor.name if nc.partition_id_tensor else None
        in_names, out_names, out_avals = [], [], []
        for alloc in nc.m.functions[0].allocations:
            if not isinstance(alloc, mybir.MemoryLocationSet):
                continue
            name = alloc.memorylocations[0].name
            if alloc.kind == "ExternalInput":
                if name != part:
                    in_names.append(name)
            elif alloc.kind == "ExternalOutput":
                out_names.append(name)
                out_avals.append(jax.core.ShapedArray(
                    tuple(alloc.tensor_shape), mybir.dt.np(alloc.dtype)))
        all_in = list(in_names) + list(out_names)
        if part is not None:
            all_in.append(part)

        def _body(*args):
            operands = list(args)
            if part is not None:
                operands.append(bass2jax.partition_id_tensor())
            return tuple(bass2jax._bass_exec_p.bind(
                *operands,
                out_avals=tuple(out_avals),
                in_names=tuple(all_in),
                out_names=tuple(out_names),
                lowering_input_output_aliases=(),
                sim_require_finite=True,
                sim_require_nnan=True,
                nc=nc,
            ))

        devices = jax.devices()[:n_cores]
        mesh = Mesh(np.asarray(devices), ("core",))
        spec = PartitionSpec("core")
        self.fn = jax.jit(
            shard_map(_body, mesh=mesh,
                      in_specs=(spec,) * (len(in_names) + len(out_names)),
                      out_specs=(spec,) * len(out_names), check_rep=False),
            keep_unused=True)
        self.mesh, self.spec = mesh, spec
        self.in_names, self.out_names, self.out_avals = \
            in_names, out_names, out_avals
        self.n_cores = n_cores
        self._dev_cache = (None, None)

    def _device_inputs(self, in_maps):
        import jax
        from jax.sharding import NamedSharding
        key = id(in_maps)
        if self._dev_cache[0] == key:
            return self._dev_cache[1]
        sh = NamedSharding(self.mesh, self.spec)
        dev_in = [jax.device_put(np.concatenate(
            [np.asarray(in_maps[c][n]) for c in range(self.n_cores)], 0), sh)
            for n in self.in_names]
        dev_zero = [jax.device_put(np.zeros(
            (self.n_cores * a.shape[0], *a.shape[1:]), a.dtype), sh)
            for a in self.out_avals]
        self._dev_cache = (key, (dev_in, dev_zero))
        return dev_in, dev_zero

    def __call__(self, in_maps):
        dev_in, dev_zero = self._device_inputs(in_maps)
        outs = self.fn(*dev_in, *dev_zero)
        self._jax.block_until_ready(outs)
        return _LazyResults(outs, self.out_names, self.out_avals,
                            self.n_cores)


class _LazyResults:
    def __init__(self, outs, out_names, out_avals, n_cores):
        self._outs, self._names, self._avals, self._n = \
            outs, out_names, out_avals, n_cores

    @property
    def results(self):
        host = [np.asarray(o) for o in self._outs]
        return [
            {n: host[i].reshape(self._n, *self._avals[i].shape)[c]
             for i, n in enumerate(self._names)}
            for c in range(self._n)
        ]


def _get_runner(iters=1):
    key = (iters, tuple(sorted(ABLATE)))
    if key not in _RUNNERS:
        _RUNNERS[key] = _CachedRunner(_get_prog(iters))
    return _RUNNERS[key]


def run_with_iters(in_maps, iters=1):
    return _get_runner(iters)(in_maps)


def host_inputs(**inputs):
    return _host_inputs(**inputs)


def _gather(results):
    out = np.empty((B, N, C), np.float32)
    for c in range(NCORES):
        bb, qs = c // 4, c % 4
        out[bb, qs * NQ:(qs + 1) * NQ, :] = results[c]["outT"].T
    return out.reshape(B, H, W, C)


def kernel(**inputs):
    in_maps = _host_inputs(**inputs)
    res = bass_utils.run_bass_kernel_spmd(_get_prog(1), in_maps,
                                          core_ids=list(range(NCORES)))
    return _gather(res.results)


# revision 31
# speedup vs baseline: 1.0810x; 1.0810x over previous
"""CrossGSA fused attention kernel for 8x Trainium2 NeuronCores.

Sharding: each core owns one batch (b = core//4) and a 576-query-token slice
(qs = core%4) of that batch, across ALL 8 heads.  k/v are computed full per
core (per-batch); the mask is the dominant traffic and is read once per
batch across the machine (each core reads mask[:, :, its q-slice],
pre-transposed on host to bf16 so DMA lines stay contiguous).

Device layouts are transposed ([channel, token]) end-to-end:
  - q/k/v/o projections run in bf16 (weights and activations converted on
    host; the f32 residual path keeps a separate f32 x-slice),
  - projections and rotary are pipelined per 512-column chunk so attention
    starts as soon as the first key tiles are rotated; rotary runs in bf16
    on the DVE (pair-swap via SBUF-SBUF DMA),
  - S^T accumulates over the head dim (K=32, 4-way tile-position packed) on
    top of the mask, which is injected into PSUM by an identity-matmul
    (the identity matmul also provides the full-row start of the PSUM
    accumulation group: a row-positioned start=True wedges the PE),
  - exp() runs on the scalar engine straight out of PSUM,
  - attn@v uses v in natural [token, dim] layout as the stationary operand,
    extended per head with a ones column so the softmax denominator falls
    out of the same matmul (no separate denominator matmuls); two heads
    pack per PSUM tile at tile_position cols {0, 64}, channels in rows
    0-31/64-95, denominators in rows 32/96; a DMA pass remaps rows to the
    channel layout afterwards,
  - the attention mt-loop is software-pipelined with a 2-tile skew
    (exp/attn@v of tile mt issue after S of tile mt+2) and the first two
    mask tiles are prefetched ahead of the bulk x DMAs,
  - the depthwise 5x5 lepe conv runs in bf16 on the DVE, overlapped with
    attention (lepe is only needed at the output projection),
  - layernorm stats use ones-matmuls; mean/rstd broadcast back via K=1
    matmuls.
The per-core [256, 576] transposed outputs are gathered and untransposed on
host.

run_with_iters() executes through a cached jax.jit callable with
device-resident inputs, so repeated calls measure device execution only
(the executable is loaded once, not per call).
"""

import numpy as np
import ml_dtypes

import concourse.bass as bass
import concourse.mybir as mybir
import concourse.tile as tile
from concourse import bacc, bass_utils

F32 = mybir.dt.float32
F32R = mybir.dt.float32r
BF16 = mybir.dt.bfloat16
F8 = mybir.dt.float8e4
AF = mybir.ActivationFunctionType
ALU = mybir.AluOpType

B, H, W, C = 2, 48, 48, 256
NH, HD = 8, 32
N = H * W            # 2304 tokens per batch
NQ = N // 4          # 576 query tokens per core
NCORES = 8
SCALING = HD ** -0.5
LN_EPS = 1e-6
MT = N // 128        # 18 key tiles
ROWS_Q = NQ // W     # 12 image rows per core
HALO = ROWS_Q + 4    # rows incl. conv halo
NHT = HALO * W       # 768 halo tokens
QCH = [(0, 512), (512, 64)]                                  # q chunks (bank)
NCH = [(0, 512), (512, 512), (1024, 512), (1536, 512), (2048, 256)]
HCH = [(0, 512), (512, 256)]

_PROGS = {}
_RUNNERS = {}
ABLATE = set()


def _bcast_ap(src, n=128):
    return bass.AP(tensor=src.tensor, offset=src.offset,
                   ap=[[0, n]] + src.ap[1:])


def _build_program(iters=1):
    nc = bacc.Bacc("TRN2", target_bir_lowering=False, debug=False,
                   enable_asserts=False, num_devices=NCORES)

    def din(name, shape, dt=F32):
        return nc.dram_tensor(name, shape, dt, kind="ExternalInput").ap()

    io = dict(
        xT=din("xT", [C, N], BF16),
        xTq=din("xTq", [C, NQ]),
        xTq_bf=din("xTq_bf", [C, NQ], BF16),
        xTh=din("xTh", [C, NHT], BF16),
        valid=din("valid", [1, NHT]),
        maskT=din("maskT", [NH, N, NQ], BF16),     # mask, transposed
        sinT=din("sinT", [HD, N], BF16),
        cosT=din("cosT", [HD, N], BF16),
        wq=din("wq", [C, C], BF16), wk=din("wk", [C, C], BF16),
        wv=din("wv", [C, C], BF16), wo=din("wo", [C, C], BF16),
        bq=din("bq", [C, 1]), bk=din("bk", [C, 1]),
        bv=din("bv", [C, 1]), bo=din("bo", [C, 1]),
        bv_row=din("bv_row", [1, C]),
        lepe_w=din("lepe_w", [C, 25]), lepe_b=din("lepe_b", [C, 1]),
        ln_g=din("ln_g", [C, 1]), ln_b=din("ln_b", [C, 1]),
        ones_p=din("ones_p", [128, 1]),
        ident_bf=din("ident_bf", [128, 128], BF16),
        ones_k1=din("ones_k1", [1, 128]),
        blockind=din("blockind", [4, 128]),
        outT=nc.dram_tensor("outT", [C, NQ], F32, kind="ExternalOutput").ap(),
    )
    with tile.TileContext(nc) as tc:
        with tc.tile_pool(name="persist", bufs=1) as P:
            for it in range(iters):
                _emit(nc, tc, P, io, it)
    nc.compile()
    return nc


def _emit(nc, tc, P, io, it=0):
    dma = nc.sync.dma_start

    def pt(tg, shape, dt=F32):
        return P.tile(shape, dt, tag=tg, name=f"{tg}_{it}")

    # ---- constants / inputs to SBUF ----
    x_sb = [pt(f"x_sb{k}", [128, N], BF16) for k in range(2)]
    xq_sb = [pt(f"xq_sb{k}", [128, NQ]) for k in range(2)]
    xqb_sb = [pt(f"xqb_sb{k}", [128, NQ], BF16) for k in range(2)]
    xh_sb = [pt(f"xh_sb{k}", [128, NHT], BF16) for k in range(2)]
    w_sb, b_sb = {}, {}
    for nm in ("wq", "wk", "wv", "wo"):
        t = pt(f"{nm}_sb", [128, 2, C], BF16)
        for kk in range(2):
            dma(t[:, kk, :], io[nm][128 * kk:128 * (kk + 1), :])
        w_sb[nm] = t
    for nm in ("bq", "bk", "bv", "bo", "lepe_b", "ln_g", "ln_b"):
        t = pt(f"{nm}_sb", [128, 2, 1])
        for kk in range(2):
            dma(t[:, kk, :], io[nm][128 * kk:128 * (kk + 1), :])
        b_sb[nm] = t
    lw_sb = pt("lw_sb", [128, 2, 25])
    for kk in range(2):
        dma(lw_sb[:, kk, :], io["lepe_w"][128 * kk:128 * (kk + 1), :])
    for k in range(2):
        for off, wd in QCH:
            dma(xqb_sb[k][:, off:off + wd],
                io["xTq_bf"][128 * k:128 * (k + 1), off:off + wd])
        dma(xq_sb[k][:], io["xTq"][128 * k:128 * (k + 1), :])
        for off, wd in NCH:
            dma(x_sb[k][:, off:off + wd],
                io["xT"][128 * k:128 * (k + 1), off:off + wd])
        dma(xh_sb[k][:], io["xTh"][128 * k:128 * (k + 1), :])
    o1_sb = pt("o1_sb", [128, 1]); dma(o1_sb[:], io["ones_p"][:])
    id_sb = pt("id_sb", [128, 128], BF16); dma(id_sb[:], io["ident_bf"][:])
    ok1_sb = pt("ok1_sb", [1, 128]); dma(ok1_sb[:], io["ones_k1"][:])
    bi_sb = pt("bi_sb", [4, 128]); dma(bi_sb[:], io["blockind"][:])
    # prefetch the first two mask tiles of group 0 ahead of the big x DMAs
    # so the attention pipeline has mask data as soon as s/exp are ready
    msk_pre = [pt(f"msk_pre{i}", [128, 4, NQ], BF16) for i in range(4)]
    mio = io["maskT"]
    for i in range(4):
        if "maskdma" not in ABLATE:
            dma(msk_pre[i][:], bass.AP(
                tensor=mio.tensor, offset=mio.offset + 128 * i * NQ,
                ap=[[NQ, 128], [N * NQ, 4], [1, NQ]]))
        else:
            nc.vector.memset(msk_pre[i][:], 1.0)
    bvr_sb = pt("bvr_sb", [128, C]); dma(bvr_sb[:], _bcast_ap(io["bv_row"]))
    val_sb = pt("val_sb", [128, NHT]); dma(val_sb[:], _bcast_ap(io["valid"]))
    sin_sb = pt("sin_sb", [128, N], BF16)
    cos_sb = pt("cos_sb", [128, N], BF16)
    dma(sin_sb[:], bass.AP(tensor=io["sinT"].tensor, offset=io["sinT"].offset,
                           ap=[[0, 4]] + io["sinT"].ap))
    dma(cos_sb[:], bass.AP(tensor=io["cosT"].tensor, offset=io["cosT"].offset,
                           ap=[[0, 4]] + io["cosT"].ap))

    kr_bf = [pt(f"kr_bf{k}", [128, N], BF16) for k in range(2)]
    qr_bf = [pt(f"qr_bf{k}", [128, NQ], BF16) for k in range(2)]
    # v extended per head: [32 ch | ones | zeros] -> 64-wide stationary blocks
    vn_bf = pt("vn_bf", [128, MT, NH, 33], BF16)
    vh_sb = [pt(f"vh_sb{k}", [128, NHT], BF16) for k in range(2)]
    vpad = [pt(f"vpad{k}", [128, HALO, W + 4], BF16) for k in range(2)]
    lepe_sb = [pt(f"lepe_sb{k}", [128, NQ], BF16) for k in range(2)]
    oat_sb = [pt(f"oat_sb{k}", [128, NQ]) for k in range(2)]
    of_sb = [pt(f"of_sb{k}", [128, NQ], BF16) for k in range(2)]
    o2_sb = [pt(f"o2_sb{k}", [128, NQ]) for k in range(2)]
    sq_sb = [pt(f"sq_sb{k}", [128, NQ]) for k in range(2)]
    ofull = [pt(f"ofull{k}", [128, 2, NQ]) for k in range(2)]
    rden4 = [pt(f"rden4{k}", [4, NQ]) for k in range(2)]
    rb_sb = pt("rb_sb", [128, NQ])
    m1_sb = pt("m1_sb", [1, NQ])
    msq_sb = pt("msq_sb", [1, NQ])
    var_sb = pt("var_sb", [1, NQ])
    rstd_sb = pt("rstd_sb", [1, NQ])
    eps_sb = pt("eps_sb", [1, 1])
    nc.vector.memset(eps_sb[:], LN_EPS)
    outf_sb = [pt(f"outf_sb{k}", [128, NQ]) for k in range(2)]

    if it == 0:
        # ones column of the v blocks; untouched by later writes, so
        # steady-state iterations skip the init.
        nc.vector.memset(vn_bf[:, :, :, 32:33], 1.0)

    # ---- Phase 1: projections + rotary, pipelined per 512-col chunk so
    # attention can start as soon as the first key tiles are rotated ----
    kTb = [pt(f"kTb{k}", [128, N], BF16) for k in range(2)]
    qTb = [pt(f"qTb{k}", [128, NQ], BF16) for k in range(2)]

    def rotary_chunk(pjs, srcT, dst, off, wd):
        # out = x*cos + pairswap(x)*sin_signed, all bf16 (2x DVE rate);
        # pair-swap partitions via SBUF->SBUF DMA (engines cannot
        # read/write strided partitions)
        shuf = pjs.tile([128, 512], BF16, tag="shuf", name="shuf")
        t1 = pjs.tile([128, 512], BF16, tag="t1", name="t1")
        dma(shuf[0:128:2, :wd], srcT[1:128:2, off:off + wd])
        dma(shuf[1:128:2, :wd], srcT[0:128:2, off:off + wd])
        nc.vector.tensor_tensor(t1[:, :wd], srcT[:, off:off + wd],
                                cos_sb[:, off:off + wd], op=ALU.mult)
        nc.vector.tensor_tensor(shuf[:, :wd], shuf[:, :wd],
                                sin_sb[:, off:off + wd], op=ALU.mult)
        nc.vector.tensor_tensor(dst[:, off:off + wd], t1[:, :wd],
                                shuf[:, :wd], op=ALU.add)

    with tc.tile_pool(name=f"pj_{it}", bufs=3, space="PSUM") as pj, \
         tc.tile_pool(name=f"pjs_{it}", bufs=6) as pjs:
        def emit_q(jt, off, wd):
            ps = pj.tile([128, 512], F32, tag="ps", name="ps")
            for kk in range(2):
                nc.tensor.matmul(ps[:, :wd],
                                 w_sb["wq"][:, kk, 128 * jt:128 * (jt + 1)],
                                 xqb_sb[kk][:, off:off + wd],
                                 start=(kk == 0), stop=(kk == 1))
            nc.vector.tensor_scalar_add(qTb[jt][:, off:off + wd],
                                        ps[:, :wd], b_sb["bq"][:, jt, :])

        def emit_k(jt, off, wd):
            ps = pj.tile([128, 512], F32, tag="ps", name="ps")
            for kk in range(2):
                nc.tensor.matmul(ps[:, :wd],
                                 w_sb["wk"][:, kk, 128 * jt:128 * (jt + 1)],
                                 x_sb[kk][:, off:off + wd],
                                 start=(kk == 0), stop=(kk == 1))
            nc.vector.tensor_scalar(kTb[jt][:, off:off + wd], ps[:, :wd],
                                    b_sb["bk"][:, jt, :], SCALING,
                                    op0=ALU.add, op1=ALU.mult)

        # v first: av(mt) only needs vn(mt), keep it off the critical chain
        for mt in range(MT):
            ps = pj.tile([128, 256], F32, tag="psv", name="psv")
            for kk in range(2):
                nc.tensor.matmul(ps[:],
                                 x_sb[kk][:, 128 * mt:128 * (mt + 1)],
                                 w_sb["wv"][:, kk, :],
                                 start=(kk == 0), stop=(kk == 1))
            nc.vector.tensor_tensor(
                vn_bf[:, mt, :, 0:32],
                ps[:].rearrange("p (h c) -> p h c", c=HD),
                bvr_sb[:].rearrange("p (h c) -> p h c", c=HD), op=ALU.add)
        # All projection matmuls + PSUM evacuations first (the evacs are the
        # last PSUM readers, so the projection banks free early for phase 2);
        # the rotary chains read only SBUF and trail behind, critical
        # (g0: kT chunk 0 + full qT) chunks first.
        for jt in range(2):
            emit_k(jt, *NCH[0])
            for off, wd in QCH:
                emit_q(jt, off, wd)
            for off, wd in NCH[1:]:
                emit_k(jt, off, wd)
        for jt in range(2):
            for off, wd in HCH:
                ps = pj.tile([128, 512], F32, tag="ps", name="ps")
                for kk in range(2):
                    nc.tensor.matmul(ps[:, :wd],
                                     w_sb["wv"][:, kk, 128 * jt:128 * (jt + 1)],
                                     xh_sb[kk][:, off:off + wd],
                                     start=(kk == 0), stop=(kk == 1))
                # vT_halo = valid*bv + psum (keeps zero-padding exact)
                nc.vector.scalar_tensor_tensor(vh_sb[jt][:, off:off + wd],
                                               val_sb[:, off:off + wd],
                                               b_sb["bv"][:, jt, :],
                                               ps[:, :wd],
                                               op0=ALU.mult, op1=ALU.add)
        if "rotary" not in ABLATE:
            for jt in range(2):
                rotary_chunk(pjs, kTb[jt], kr_bf[jt], *NCH[0])
                for off, wd in QCH:
                    rotary_chunk(pjs, qTb[jt], qr_bf[jt], off, wd)
                for off, wd in NCH[1:]:
                    rotary_chunk(pjs, kTb[jt], kr_bf[jt], off, wd)

    if "rotary" in ABLATE:
        for jt in range(2):
            nc.vector.memset(kr_bf[jt][:], 0.01)
            nc.vector.memset(qr_bf[jt][:], 0.01)

    # ---- Phase 1b: depthwise 5x5 lepe conv on GPSIMD ----
    if "conv" in ABLATE:
        for jt in range(2):
            nc.vector.memset(lepe_sb[jt][:], 0.0)
    for jt in range(2) if "conv" not in ABLATE else []:
        nc.gpsimd.memset(vpad[jt][:], 0.0)
        nc.gpsimd.tensor_copy(
            vpad[jt][:, :, 2:2 + W],
            vh_sb[jt][:].rearrange("p (r w) -> p r w", w=W))
        lp = lepe_sb[jt][:].rearrange("p (r w) -> p r w", w=W)
        first = True
        for dy in range(5):
            for dx in range(5):
                src = vpad[jt][:, dy:dy + ROWS_Q, dx:dx + W]
                wtap = lw_sb[:, jt, 5 * dy + dx:5 * dy + dx + 1]
                if first:
                    nc.vector.tensor_scalar(lp, src, wtap,
                                            b_sb["lepe_b"][:, jt, :],
                                            op0=ALU.mult, op1=ALU.add)
                    first = False
                else:
                    nc.vector.scalar_tensor_tensor(lp, src, wtap, lp,
                                                   op0=ALU.mult, op1=ALU.add)

    # ---- Phase 2: attention, two 4-head supergroups ----
    for g in range(2):
        with tc.tile_pool(name=f"op{g}_{it}", bufs=1, space="PSUM") as op:
            o_t = [op.tile([128, 512], F32, tag=f"o_t{pr}", name=f"o_t{pr}")
                   for pr in range(2)]
            o_s = op.tile([128, 2, 64], F32, tag="o_s", name="o_s")
            with tc.tile_pool(name=f"sp{g}_{it}", bufs=2, space="PSUM") as sp, \
                 tc.tile_pool(name=f"stp{g}_{it}", bufs=1, space="PSUM") as stp, \
                 tc.tile_pool(name=f"mp{g}_{it}", bufs=4) as mp, \
                 tc.tile_pool(name=f"pp{g}_{it}", bufs=4) as pp:
                live = {}

                def emit_s(mt):
                    if g == 0 and mt < 4:
                        msk = msk_pre[mt]
                    else:
                        msk = mp.tile([128, 4, NQ], BF16, tag="msk",
                                      name="msk")
                        mio = io["maskT"]
                        src = bass.AP(
                            tensor=mio.tensor,
                            offset=mio.offset + (4 * g) * N * NQ
                            + 128 * mt * NQ,
                            ap=[[NQ, 128], [N * NQ, 4], [1, NQ]])
                        if "maskdma" not in ABLATE:
                            dma(msk[:], src)
                        else:
                            nc.vector.memset(msk[:], 1.0)
                    s_pair = [sp.tile([128, 2, 512], F32, tag="s", name="s")
                              for _ in range(2)]
                    stub = stp.tile([128, 4, 64], F32, tag="stub", name="stub")
                    for j in range(4) if "smm" not in ABLATE else []:
                        pr, ln_ = j // 2, j % 2
                        lhs = kr_bf[g][32 * j:32 * (j + 1),
                                       128 * mt:128 * (mt + 1)]
                        rq = qr_bf[g]
                        # inject mask via identity matmul: provides the
                        # full-row start of the PSUM group (a row-positioned
                        # start=True wedges the PE on this hardware)
                        nc.tensor.matmul(s_pair[pr][:, ln_, :], id_sb[:],
                                         msk[:, j, 0:512],
                                         start=True, stop=False)
                        nc.tensor.matmul(s_pair[pr][:, ln_, :], lhs,
                                         rq[32 * j:32 * (j + 1), 0:512],
                                         start=False, stop=True,
                                         tile_position=(32 * j, 0))
                        nc.tensor.matmul(stub[:, j, :], id_sb[:],
                                         msk[:, j, 512:576],
                                         start=True, stop=False)
                        nc.tensor.matmul(stub[:, j, :], lhs,
                                         rq[32 * j:32 * (j + 1), 512:576],
                                         start=False, stop=True,
                                         tile_position=(32 * j, 0))
                    if "smm" in ABLATE:
                        for pr in range(2):
                            nc.vector.memset(s_pair[pr][:], 0.01)
                        nc.vector.memset(stub[:], 0.01)
                    live[mt] = (msk, s_pair, stub)

                def emit_epa(mt):
                    # exp + mask-multiply + attn@v for tile mt; issued after
                    # s(mt+1) so the PE stream never stalls on the Act/DVE
                    # chain of the current tile.
                    msk, s_pair, stub = live.pop(mt)
                    p_sb = pp.tile([128, 4, NQ], BF16, tag="p_sb", name="p_sb")
                    if "exp" in ABLATE:
                        nc.vector.memset(p_sb[:], 0.001)
                    if "exp" not in ABLATE:
                        for pr in range(2):
                            nc.scalar.activation(p_sb[:, 2 * pr:2 * pr + 2, 0:512],
                                                 s_pair[pr][:], AF.Exp)
                        nc.scalar.activation(p_sb[:, :, 512:576], stub[:], AF.Exp)
                    for j in range(4) if "av" not in ABLATE else []:
                        pr, ln_ = j // 2, j % 2
                        h = 4 * g + j
                        lhsv = vn_bf[:, mt, h, :]
                        nc.tensor.matmul(o_t[pr][64 * ln_:64 * ln_ + 33, :],
                                         lhsv, p_sb[:, j, 0:512],
                                         start=(mt == 0), stop=(mt == MT - 1),
                                         tile_position=(0, 64 * ln_))
                        nc.tensor.matmul(o_s[64 * ln_:64 * ln_ + 33, pr, :],
                                         lhsv, p_sb[:, j, 512:576],
                                         start=(mt == 0), stop=(mt == MT - 1),
                                         tile_position=(0, 64 * ln_))

                if "noskew" in ABLATE:
                    for mt in range(MT):
                        emit_s(mt)
                        emit_epa(mt)
                else:
                    for mt in range(MT):
                        emit_s(mt)
                        if mt > 1:
                            emit_epa(mt - 2)
                    emit_epa(MT - 2)
                    emit_epa(MT - 1)
            if "av" in ABLATE:
                nc.vector.memset(oat_sb[g][:], 0.01)
                nc.vector.tensor_tensor(of_sb[g][:], oat_sb[g][:],
                                        lepe_sb[g][:], op=ALU.add)
                continue
            # evacuate lane-preserving, then DMA-remap rows:
            # head j=2*pr+ln at psum rows 64*ln..64*ln+32 of tile pr,
            # denominator at row 64*ln+32.
            for pr in range(2):
                nc.vector.tensor_copy(ofull[g][:, pr, 0:512], o_t[pr][:])
                nc.vector.tensor_copy(ofull[g][:, pr, 512:576], o_s[:, pr, :])
            for pr in range(2):
                dma(rden4[g][2 * pr:2 * pr + 2, :],
                    ofull[g][32:128:64, pr, :])
            for j in range(4):
                pr, ln_ = j // 2, j % 2
                dma(oat_sb[g][32 * j:32 * (j + 1), :],
                    ofull[g][64 * ln_:64 * ln_ + 32, pr, :])
            nc.vector.reciprocal(rden4[g][:], rden4[g][:])
            with tc.tile_pool(name=f"rb{g}_{it}", bufs=1, space="PSUM") as rbp:
                rb_m = rbp.tile([128, 512], F32, tag="rb_m", name="rb_m")
                rb_s = rbp.tile([128, 64], F32, tag="rb_s", name="rb_s")
                nc.tensor.matmul(rb_m[:], bi_sb[:], rden4[g][:, 0:512],
                                 start=True, stop=True)
                nc.tensor.matmul(rb_s[:], bi_sb[:], rden4[g][:, 512:576],
                                 start=True, stop=True)
                nc.vector.tensor_copy(rb_sb[:, 0:512], rb_m[:])
                nc.vector.tensor_copy(rb_sb[:, 512:576], rb_s[:])
            nc.vector.tensor_tensor(oat_sb[g][:], oat_sb[g][:], rb_sb[:],
                                    op=ALU.mult)
        nc.vector.tensor_tensor(of_sb[g][:], oat_sb[g][:], lepe_sb[g][:],
                                op=ALU.add)

    # ---- Phase 3: out-projection + residual + layernorm ----
    with tc.tile_pool(name=f"pwp_{it}", bufs=2, space="PSUM") as pwp:
        for jt in range(2):
            for off, wd in QCH:
                ps = pwp.tile([128, 512], F32, tag="pw", name="pw")
                for kk in range(2):
                    nc.tensor.matmul(ps[:, :wd],
                                     w_sb["wo"][:, kk, 128 * jt:128 * (jt + 1)],
                                     of_sb[kk][:, off:off + wd],
                                     start=(kk == 0), stop=(kk == 1))
                nc.vector.scalar_tensor_tensor(o2_sb[jt][:, off:off + wd],
                                               ps[:, :wd],
                                               b_sb["bo"][:, jt, :],
                                               xq_sb[jt][:, off:off + wd],
                                               op0=ALU.add, op1=ALU.add)
            nc.vector.tensor_tensor(sq_sb[jt][:], o2_sb[jt][:], o2_sb[jt][:],
                                    op=ALU.mult)
    with tc.tile_pool(name=f"stat_{it}", bufs=1, space="PSUM") as st:
        mu, ssq = {}, {}
        for off, wd in QCH:
            mu[off] = st.tile([1, wd], F32, tag=f"mu{off}", name="mu")
            ssq[off] = st.tile([1, wd], F32, tag=f"ssq{off}", name="ssq")
            for jt in range(2):
                nc.tensor.matmul(mu[off][:], o1_sb[:],
                                 o2_sb[jt][:, off:off + wd],
                                 start=(jt == 0), stop=(jt == 1))
                nc.tensor.matmul(ssq[off][:], o1_sb[:],
                                 sq_sb[jt][:, off:off + wd],
                                 start=(jt == 0), stop=(jt == 1))
        for off, wd in QCH:
            sl = slice(off, off + wd)
            nc.vector.tensor_scalar_mul(m1_sb[:, sl], mu[off][:], 1.0 / C)
            nc.vector.tensor_tensor(msq_sb[:, sl], m1_sb[:, sl],
                                    m1_sb[:, sl], op=ALU.mult)
            nc.vector.scalar_tensor_tensor(var_sb[:, sl], ssq[off][:],
                                           1.0 / C, msq_sb[:, sl],
                                           op0=ALU.mult, op1=ALU.subtract)
        nc.scalar.activation(rstd_sb[:], var_sb[:], AF.Sqrt, bias=eps_sb[:])
        nc.vector.reciprocal(rstd_sb[:], rstd_sb[:])
    with tc.tile_pool(name=f"bc_{it}", bufs=1, space="PSUM") as bc:
        mb = bc.tile([128, 512], F32, tag="mb", name="mb")
        mbs = bc.tile([128, 64], F32, tag="mbs", name="mbs")
        rbm = bc.tile([128, 512], F32, tag="rbm", name="rbm")
        rbs = bc.tile([128, 64], F32, tag="rbs", name="rbs")
        nc.tensor.matmul(mb[:], ok1_sb[:], m1_sb[:, 0:512],
                         start=True, stop=True)
        nc.tensor.matmul(mbs[:], ok1_sb[:], m1_sb[:, 512:576],
                         start=True, stop=True)
        nc.tensor.matmul(rbm[:], ok1_sb[:], rstd_sb[:, 0:512],
                         start=True, stop=True)
        nc.tensor.matmul(rbs[:], ok1_sb[:], rstd_sb[:, 512:576],
                         start=True, stop=True)
        mb_sb = sq_sb[0]  # scratch reuse
        rs_sb = rb_sb
        nc.vector.tensor_copy(mb_sb[:, 0:512], mb[:])
        nc.vector.tensor_copy(mb_sb[:, 512:576], mbs[:])
        nc.vector.tensor_copy(rs_sb[:, 0:512], rbm[:])
        nc.vector.tensor_copy(rs_sb[:, 512:576], rbs[:])
    for jt in range(2):
        t1 = oat_sb[jt]  # scratch reuse
        nc.vector.tensor_tensor(t1[:], o2_sb[jt][:], mb_sb[:],
                                op=ALU.subtract)
        nc.vector.tensor_tensor(t1[:], t1[:], rs_sb[:], op=ALU.mult)
        nc.vector.affine_then_add(outf_sb[jt][:], t1[:], o2_sb[jt][:],
                                  b_sb["ln_g"][:, jt, :],
                                  b_sb["ln_b"][:, jt, :])
        dma(io["outT"][128 * jt:128 * (jt + 1), :], outf_sb[jt][:])


def _host_inputs(x, sin, cos, mask, wq, bq, wk, bk, wv, bv,
                 lepe_w, lepe_b, wo, bo, ln_g, ln_b):
    bf = ml_dtypes.bfloat16
    maskT = np.ascontiguousarray(
        np.transpose(np.asarray(mask, np.float32), (0, 2, 1))).astype(bf)
    pm1 = np.tile(np.array([-1.0, 1.0], np.float32), HD // 2).reshape(HD, 1)
    sinT = np.ascontiguousarray(
        (np.asarray(sin, np.float32).reshape(N, HD).T * pm1).astype(bf))
    cosT = np.ascontiguousarray(
        np.asarray(cos, np.float32).reshape(N, HD).T.astype(bf))
    col = lambda a: np.asarray(a, np.float32).reshape(C, 1)
    common = dict(
        sinT=sinT, cosT=cosT,
        wq=np.asarray(wq, np.float32).astype(bf),
        wk=np.asarray(wk, np.float32).astype(bf),
        wv=np.asarray(wv, np.float32).astype(bf),
        wo=np.asarray(wo, np.float32).astype(bf),
        bq=col(bq), bk=col(bk), bv=col(bv), bo=col(bo),
        bv_row=np.asarray(bv, np.float32).reshape(1, C),
        lepe_w=np.asarray(lepe_w, np.float32).reshape(C, 25),
        lepe_b=col(lepe_b), ln_g=col(ln_g), ln_b=col(ln_b),
        ones_p=np.ones((128, 1), np.float32),
        ident_bf=np.eye(128, dtype=bf),
        ones_k1=np.ones((1, 128), np.float32),
        blockind=np.repeat(np.eye(4, dtype=np.float32), 32, axis=1
                           ).reshape(4, 128),
    )
    xs = np.asarray(x, np.float32).reshape(B, N, C)
    in_maps = []
    for c in range(NCORES):
        bb, qs = c // 4, c % 4
        xTb = np.ascontiguousarray(xs[bb].T)
        q0 = qs * NQ
        r0 = qs * ROWS_Q - 2
        halo = np.zeros((C, NHT), bf)
        vmask = np.zeros((1, NHT), np.float32)
        for r in range(HALO):
            ri = r0 + r
            if 0 <= ri < H:
                halo[:, r * W:(r + 1) * W] = xTb[:, ri * W:(ri + 1) * W].astype(bf)
                vmask[0, r * W:(r + 1) * W] = 1.0
        xq_f = np.ascontiguousarray(xTb[:, q0:q0 + NQ])
        m = dict(common)
        m.update(xT=xTb.astype(bf), xTq=xq_f, xTq_bf=xq_f.astype(bf),
                 xTh=halo, valid=vmask,
                 maskT=np.ascontiguousarray(maskT[:, :, q0:q0 + NQ]))
        in_maps.append(m)
    return in_maps


def _get_prog(iters=1):
    key = (iters, tuple(sorted(ABLATE)))
    if key not in _PROGS:
        _PROGS[key] = _build_program(iters)
    return _PROGS[key]


class _CachedRunner:
    """jax.jit callable built once per program; inputs device-put once per
    in_maps object, so repeated calls measure device execution only."""

    def __init__(self, nc, n_cores=NCORES):
        import jax
        from jax.sharding import Mesh, PartitionSpec
        from jax.experimental.shard_map import shard_map
        from concourse import bass2jax

        bass2jax.install_neuronx_cc_hook()
        self._jax = jax
        part = nc.partition_id_tensor.name if nc.partition_id_tensor else None
        in_names, out_names, out_avals = [], [], []
        for alloc in nc.m.functions[0].allocations:
            if not isinstance(alloc, mybir.MemoryLocationSet):
                continue
            name = alloc.memorylocations[0].name
            if alloc.kind == "ExternalInput":
                if name != part:
                    in_names.append(name)
            elif alloc.kind == "ExternalOutput":
                out_names.append(name)
                out_avals.append(jax.core.ShapedArray(
                    tuple(alloc.tensor_shape), mybir.dt.np(alloc.dtype)))
        all_in = list(in_names) + list(out_names)
        if part is not None:
            all_in.append(part)

        def _body(*args):
            operands = list(args)
            if part is not None:
                operands.append(bass2jax.partition_id_tensor())
            return tuple(bass2jax._bass_exec_p.bind(
                *operands,
                out_avals=tuple(out_avals),
                in_names=tuple(all_in),
                out_names=tuple(out_names),
                lowering_input_output_aliases=(),
                sim_require_finite=True,
                sim_require_nnan=True,
                nc=nc,
            ))

        devices = jax.devices()[:n_cores]
        mesh = Mesh(np.asarray(devices), ("core",))
        spec = PartitionSpec("core")
        self.fn = jax.jit(
            shard_map(_body, mesh=mesh,
                      in_specs=(spec,) * (len(in_names) + len(out_names)),
                      out_specs=(spec,) * len(out_names), check_rep=False),
            keep_unused=True)
        self.mesh, self.spec = mesh, spec
        self.in_names, self.out_names, self.out_avals = \
            in_names, out_names, out_avals
        self.n_cores = n_cores
        self._dev_cache = (None, None)

    def _device_inputs(self, in_maps):
        import jax
        from jax.sharding import NamedSharding
        key = id(in_maps)
        if self._dev_cache[0] == key:
            return self._dev_cache[1]
        sh = NamedSharding(self.mesh, self.spec)
        dev_in = [jax.device_put(np.concatenate(
            [np.asarray(in_maps[c][n]) for c in range(self.n_cores)], 0), sh)
            for n in self.in_names]
        dev_zero = [jax.device_put(np.zeros(
            (self.n_cores * a.shape[0], *a.shape[1:]), a.dtype), sh)
            for a in self.out_avals]
        self._dev_cache = (key, (dev_in, dev_zero))
        return dev_in, dev_zero

    def __call__(self, in_maps):
        dev_in, dev_zero = self._device_inputs(in_maps)
        outs = self.fn(*dev_in, *dev_zero)
        self._jax.block_until_ready(outs)
        return _LazyResults(outs, self.out_names, self.out_avals,
                            self.n_cores)


class _LazyResults:
    def __init__(self, outs, out_names, out_avals, n_cores):
        self._outs, self._names, self._avals, self._n = \
            outs, out_names, out_avals, n_cores

    @property
    def results(self):
        host = [np.asarray(o) for o in self._outs]
        return [
            {n: host[i].reshape(self._n, *self._avals[i].shape)[c]
             for i, n in enumerate(self._names)}
            for c in range(self._n)
        ]


def _get_runner(iters=1):
    key = (iters, tuple(sorted(ABLATE)))
    if key not in _RUNNERS:
        _RUNNERS[key] = _CachedRunner(_get_prog(iters))
    return _RUNNERS[key]


def run_with_iters(in_maps, iters=1):
    return _get_runner(iters)(in_maps)


def host_inputs(**inputs):
    return _host_inputs(**inputs)


def _gather(results):
    out = np.empty((B, N, C), np.float32)
    for c in range(NCORES):
        bb, qs = c // 4, c % 4
        out[bb, qs * NQ:(qs + 1) * NQ, :] = results[c]["outT"].T
    return out.reshape(B, H, W, C)


def kernel(**inputs):
    in_maps = _host_inputs(**inputs)
    res = bass_utils.run_bass_kernel_spmd(_get_prog(1), in_maps,
                                          core_ids=list(range(NCORES)))
    return _gather(res.results)


# revision 32
# speedup vs baseline: 1.7197x; 1.5908x over previous
"""CrossGSA fused attention kernel for 8x Trainium2 NeuronCores.

Sharding: each core owns one batch (b = core//4) and a 576-query-token slice
(qs = core%4) of that batch, across ALL 8 heads.  k/v are computed full per
core (per-batch); the mask is the dominant traffic and is read once per
batch across the machine (each core reads mask[:, :, its q-slice],
pre-transposed on host to bf16 so DMA lines stay contiguous).

Device layouts are transposed ([channel, token]) end-to-end:
  - q/k/v/o projections run in bf16 (weights and activations converted on
    host; the f32 residual path keeps a separate f32 x-slice),
  - projections and rotary are pipelined per 512-column chunk so attention
    starts as soon as the first key tiles are rotated; rotary runs in bf16
    on the DVE (pair-swap via SBUF-SBUF DMA),
  - S^T accumulates over the head dim (K=32, 4-way tile-position packed) on
    top of the mask, which is injected into PSUM by an identity-matmul
    (the identity matmul also provides the full-row start of the PSUM
    accumulation group: a row-positioned start=True wedges the PE),
  - exp() runs on the scalar engine straight out of PSUM,
  - attn@v uses v in natural [token, dim] layout as the stationary operand,
    extended per head with a ones column so the softmax denominator falls
    out of the same matmul (no separate denominator matmuls); two heads
    pack per PSUM tile at tile_position cols {0, 64}, channels in rows
    0-31/64-95, denominators in rows 32/96; a DMA pass remaps rows to the
    channel layout afterwards,
  - the attention mt-loop is software-pipelined with a 2-tile skew
    (exp/attn@v of tile mt issue after S of tile mt+2) and the first two
    mask tiles are prefetched ahead of the bulk x DMAs,
  - the depthwise 5x5 lepe conv runs in bf16 on the DVE, overlapped with
    attention (lepe is only needed at the output projection),
  - layernorm stats use ones-matmuls; mean/rstd broadcast back via K=1
    matmuls.
The per-core [256, 576] transposed outputs are gathered and untransposed on
host.

run_with_iters() executes through a cached jax.jit callable with
device-resident inputs, so repeated calls measure device execution only
(the executable is loaded once, not per call).
"""

import numpy as np
import ml_dtypes

import concourse.bass as bass
import concourse.mybir as mybir
import concourse.tile as tile
from concourse import bacc, bass_utils

F32 = mybir.dt.float32
F32R = mybir.dt.float32r
BF16 = mybir.dt.bfloat16
F8 = mybir.dt.float8e4
AF = mybir.ActivationFunctionType
ALU = mybir.AluOpType

B, H, W, C = 2, 48, 48, 256
NH, HD = 8, 32
N = H * W            # 2304 tokens per batch
NQ = N // 4          # 576 query tokens per core
NCORES = 8
SCALING = HD ** -0.5
LN_EPS = 1e-6
MT = N // 128        # 18 key tiles
ROWS_Q = NQ // W     # 12 image rows per core
HALO = ROWS_Q + 4    # rows incl. conv halo
NHT = HALO * W       # 768 halo tokens
QCH = [(0, 512), (512, 64)]                                  # q chunks (bank)
NCH = [(0, 512), (512, 512), (1024, 512), (1536, 512), (2048, 256)]
HCH = [(0, 512), (512, 256)]

_PROGS = {}
_RUNNERS = {}
ABLATE = set()


def _bcast_ap(src, n=128):
    return bass.AP(tensor=src.tensor, offset=src.offset,
                   ap=[[0, n]] + src.ap[1:])


def _build_program(iters=1):
    nc = bacc.Bacc("TRN2", target_bir_lowering=False, debug=False,
                   enable_asserts=False, num_devices=NCORES)

    def din(name, shape, dt=F32):
        return nc.dram_tensor(name, shape, dt, kind="ExternalInput").ap()

    io = dict(
        xT=din("xT", [C, N], BF16),
        xTq=din("xTq", [C, NQ]),
        xTq_bf=din("xTq_bf", [C, NQ], BF16),
        xTh=din("xTh", [C, NHT], BF16),
        valid=din("valid", [1, NHT]),
        maskT=din("maskT", [NH, N, NQ], BF16),     # mask, transposed
        sinT=din("sinT", [HD, N], BF16),
        cosT=din("cosT", [HD, N], BF16),
        wq=din("wq", [C, C], BF16), wk=din("wk", [C, C], BF16),
        wv=din("wv", [C, C], BF16), wo=din("wo", [C, C], BF16),
        bq=din("bq", [C, 1]), bk=din("bk", [C, 1]),
        bv=din("bv", [C, 1]), bo=din("bo", [C, 1]),
        bv_row=din("bv_row", [1, C]),
        lepe_w=din("lepe_w", [C, 25]), lepe_b=din("lepe_b", [C, 1]),
        ln_g=din("ln_g", [C, 1]), ln_b=din("ln_b", [C, 1]),
        ones_p=din("ones_p", [128, 1]),
        ident_bf=din("ident_bf", [128, 128], BF16),
        ones_k1=din("ones_k1", [1, 128]),
        blockind=din("blockind", [4, 128]),
        outT=nc.dram_tensor("outT", [C, NQ], F32, kind="ExternalOutput").ap(),
    )
    with tile.TileContext(nc) as tc:
        with tc.tile_pool(name="persist", bufs=1) as P:
            for it in range(iters):
                _emit(nc, tc, P, io, it)
    nc.compile()
    return nc


def _emit(nc, tc, P, io, it=0):
    dma = nc.sync.dma_start

    def pt(tg, shape, dt=F32):
        return P.tile(shape, dt, tag=tg, name=f"{tg}_{it}")

    # ---- constants / inputs to SBUF ----
    x_sb = [pt(f"x_sb{k}", [128, N], BF16) for k in range(2)]
    xq_sb = [pt(f"xq_sb{k}", [128, NQ]) for k in range(2)]
    xqb_sb = [pt(f"xqb_sb{k}", [128, NQ], BF16) for k in range(2)]
    xh_sb = [pt(f"xh_sb{k}", [128, NHT], BF16) for k in range(2)]
    w_sb, b_sb = {}, {}
    for nm in ("wq", "wk", "wv", "wo"):
        t = pt(f"{nm}_sb", [128, 2, C], BF16)
        for kk in range(2):
            dma(t[:, kk, :], io[nm][128 * kk:128 * (kk + 1), :])
        w_sb[nm] = t
    for nm in ("bq", "bk", "bv", "bo", "lepe_b", "ln_g", "ln_b"):
        t = pt(f"{nm}_sb", [128, 2, 1])
        for kk in range(2):
            dma(t[:, kk, :], io[nm][128 * kk:128 * (kk + 1), :])
        b_sb[nm] = t
    lw_sb = pt("lw_sb", [128, 2, 25])
    for kk in range(2):
        dma(lw_sb[:, kk, :], io["lepe_w"][128 * kk:128 * (kk + 1), :])
    for k in range(2):
        for off, wd in QCH:
            dma(xqb_sb[k][:, off:off + wd],
                io["xTq_bf"][128 * k:128 * (k + 1), off:off + wd])
        dma(xq_sb[k][:], io["xTq"][128 * k:128 * (k + 1), :])
        for off, wd in NCH:
            dma(x_sb[k][:, off:off + wd],
                io["xT"][128 * k:128 * (k + 1), off:off + wd])
        dma(xh_sb[k][:], io["xTh"][128 * k:128 * (k + 1), :])
    o1_sb = pt("o1_sb", [128, 1]); dma(o1_sb[:], io["ones_p"][:])
    id_sb = pt("id_sb", [128, 128], BF16); dma(id_sb[:], io["ident_bf"][:])
    ok1_sb = pt("ok1_sb", [1, 128]); dma(ok1_sb[:], io["ones_k1"][:])
    bi_sb = pt("bi_sb", [4, 128]); dma(bi_sb[:], io["blockind"][:])
    # prefetch the first two mask tiles of group 0 ahead of the big x DMAs
    # so the attention pipeline has mask data as soon as s/exp are ready
    msk_pre = [pt(f"msk_pre{i}", [128, 4, NQ], BF16) for i in range(6)]
    mio = io["maskT"]
    for i in range(6):
        if "maskdma" not in ABLATE:
            dma(msk_pre[i][:], bass.AP(
                tensor=mio.tensor, offset=mio.offset + 128 * i * NQ,
                ap=[[NQ, 128], [N * NQ, 4], [1, NQ]]))
        else:
            nc.vector.memset(msk_pre[i][:], 1.0)
    bvr_sb = pt("bvr_sb", [128, C]); dma(bvr_sb[:], _bcast_ap(io["bv_row"]))
    val_sb = pt("val_sb", [128, NHT]); dma(val_sb[:], _bcast_ap(io["valid"]))
    sin_sb = pt("sin_sb", [128, N], BF16)
    cos_sb = pt("cos_sb", [128, N], BF16)
    dma(sin_sb[:], bass.AP(tensor=io["sinT"].tensor, offset=io["sinT"].offset,
                           ap=[[0, 4]] + io["sinT"].ap))
    dma(cos_sb[:], bass.AP(tensor=io["cosT"].tensor, offset=io["cosT"].offset,
                           ap=[[0, 4]] + io["cosT"].ap))

    kr_bf = [pt(f"kr_bf{k}", [128, N], BF16) for k in range(2)]
    qr_bf = [pt(f"qr_bf{k}", [128, NQ], BF16) for k in range(2)]
    # v extended per head: [32 ch | ones | zeros] -> 64-wide stationary blocks
    vn_bf = pt("vn_bf", [128, MT, NH, 33], BF16)
    vh_sb = [pt(f"vh_sb{k}", [128, NHT], BF16) for k in range(2)]
    vpad = [pt(f"vpad{k}", [128, HALO, W + 4], BF16) for k in range(2)]
    lepe_sb = [pt(f"lepe_sb{k}", [128, NQ], BF16) for k in range(2)]
    oat_sb = [pt(f"oat_sb{k}", [128, NQ]) for k in range(2)]
    of_sb = [pt(f"of_sb{k}", [128, NQ], BF16) for k in range(2)]
    o2_sb = [pt(f"o2_sb{k}", [128, NQ]) for k in range(2)]
    sq_sb = [pt(f"sq_sb{k}", [128, NQ]) for k in range(2)]
    ofull = [pt(f"ofull{k}", [128, 2, NQ]) for k in range(2)]
    rden4 = [pt(f"rden4{k}", [4, NQ]) for k in range(2)]
    rb_sb = pt("rb_sb", [128, NQ])
    m1_sb = pt("m1_sb", [1, NQ])
    msq_sb = pt("msq_sb", [1, NQ])
    var_sb = pt("var_sb", [1, NQ])
    rstd_sb = pt("rstd_sb", [1, NQ])
    eps_sb = pt("eps_sb", [1, 1])
    nc.vector.memset(eps_sb[:], LN_EPS)
    outf_sb = [pt(f"outf_sb{k}", [128, NQ]) for k in range(2)]

    if it == 0:
        # ones column of the v blocks; untouched by later writes, so
        # steady-state iterations skip the init.
        nc.vector.memset(vn_bf[:, :, :, 32:33], 1.0)

    # ---- Phase 1: projections + rotary, pipelined per 512-col chunk so
    # attention can start as soon as the first key tiles are rotated ----
    kTb = [pt(f"kTb{k}", [128, N], BF16) for k in range(2)]
    qTb = [pt(f"qTb{k}", [128, NQ], BF16) for k in range(2)]

    def rotary_chunk(pjs, srcT, dst, off, wd):
        # out = x*cos + pairswap(x)*sin_signed, all bf16 (2x DVE rate);
        # pair-swap partitions via SBUF->SBUF DMA (engines cannot
        # read/write strided partitions)
        shuf = pjs.tile([128, 512], BF16, tag="shuf", name="shuf")
        t1 = pjs.tile([128, 512], BF16, tag="t1", name="t1")
        dma(shuf[0:128:2, :wd], srcT[1:128:2, off:off + wd])
        dma(shuf[1:128:2, :wd], srcT[0:128:2, off:off + wd])
        nc.vector.tensor_tensor(t1[:, :wd], srcT[:, off:off + wd],
                                cos_sb[:, off:off + wd], op=ALU.mult)
        nc.vector.tensor_tensor(shuf[:, :wd], shuf[:, :wd],
                                sin_sb[:, off:off + wd], op=ALU.mult)
        nc.vector.tensor_tensor(dst[:, off:off + wd], t1[:, :wd],
                                shuf[:, :wd], op=ALU.add)

    with tc.tile_pool(name=f"pj_{it}", bufs=3, space="PSUM") as pj, \
         tc.tile_pool(name=f"pjs_{it}", bufs=8) as pjs:
        def emit_q(jt, off, wd):
            ps = pj.tile([128, 512], F32, tag="ps", name="ps")
            for kk in range(2):
                nc.tensor.matmul(ps[:, :wd],
                                 w_sb["wq"][:, kk, 128 * jt:128 * (jt + 1)],
                                 xqb_sb[kk][:, off:off + wd],
                                 start=(kk == 0), stop=(kk == 1))
            nc.vector.tensor_scalar_add(qTb[jt][:, off:off + wd],
                                        ps[:, :wd], b_sb["bq"][:, jt, :])

        def emit_k(jt, off, wd):
            ps = pj.tile([128, 512], F32, tag="ps", name="ps")
            for kk in range(2):
                nc.tensor.matmul(ps[:, :wd],
                                 w_sb["wk"][:, kk, 128 * jt:128 * (jt + 1)],
                                 x_sb[kk][:, off:off + wd],
                                 start=(kk == 0), stop=(kk == 1))
            nc.vector.tensor_scalar(kTb[jt][:, off:off + wd], ps[:, :wd],
                                    b_sb["bk"][:, jt, :], SCALING,
                                    op0=ALU.add, op1=ALU.mult)

        # v first: av(mt) only needs vn(mt), keep it off the critical chain
        for mt in range(MT):
            ps = pj.tile([128, 256], F32, tag="psv", name="psv")
            for kk in range(2):
                nc.tensor.matmul(ps[:],
                                 x_sb[kk][:, 128 * mt:128 * (mt + 1)],
                                 w_sb["wv"][:, kk, :],
                                 start=(kk == 0), stop=(kk == 1))
            nc.vector.tensor_tensor(
                vn_bf[:, mt, :, 0:32],
                ps[:].rearrange("p (h c) -> p h c", c=HD),
                bvr_sb[:].rearrange("p (h c) -> p h c", c=HD), op=ALU.add)
        # All projection matmuls + PSUM evacuations first (the evacs are the
        # last PSUM readers, so the projection banks free early for phase 2);
        # the rotary chains read only SBUF and trail behind, critical
        # (g0: kT chunk 0 + full qT) chunks first.
        for jt in range(2):
            emit_k(jt, *NCH[0])
            for off, wd in QCH:
                emit_q(jt, off, wd)
            for off, wd in NCH[1:]:
                emit_k(jt, off, wd)
        for jt in range(2):
            for off, wd in HCH:
                ps = pj.tile([128, 512], F32, tag="ps", name="ps")
                for kk in range(2):
                    nc.tensor.matmul(ps[:, :wd],
                                     w_sb["wv"][:, kk, 128 * jt:128 * (jt + 1)],
                                     xh_sb[kk][:, off:off + wd],
                                     start=(kk == 0), stop=(kk == 1))
                # vT_halo = valid*bv + psum (keeps zero-padding exact)
                nc.vector.scalar_tensor_tensor(vh_sb[jt][:, off:off + wd],
                                               val_sb[:, off:off + wd],
                                               b_sb["bv"][:, jt, :],
                                               ps[:, :wd],
                                               op0=ALU.mult, op1=ALU.add)
        if "rotary" not in ABLATE:
            for jt in range(2):
                rotary_chunk(pjs, kTb[jt], kr_bf[jt], *NCH[0])
                for off, wd in QCH:
                    rotary_chunk(pjs, qTb[jt], qr_bf[jt], off, wd)
                for off, wd in NCH[1:]:
                    rotary_chunk(pjs, kTb[jt], kr_bf[jt], off, wd)

    if "rotary" in ABLATE:
        for jt in range(2):
            nc.vector.memset(kr_bf[jt][:], 0.01)
            nc.vector.memset(qr_bf[jt][:], 0.01)

    # ---- Phase 1b: depthwise 5x5 lepe conv on GPSIMD ----
    if "conv" in ABLATE:
        for jt in range(2):
            nc.vector.memset(lepe_sb[jt][:], 0.0)
    for jt in range(2) if "conv" not in ABLATE else []:
        nc.gpsimd.memset(vpad[jt][:], 0.0)
        nc.gpsimd.tensor_copy(
            vpad[jt][:, :, 2:2 + W],
            vh_sb[jt][:].rearrange("p (r w) -> p r w", w=W))
        lp = lepe_sb[jt][:].rearrange("p (r w) -> p r w", w=W)
        first = True
        for dy in range(5):
            for dx in range(5):
                src = vpad[jt][:, dy:dy + ROWS_Q, dx:dx + W]
                wtap = lw_sb[:, jt, 5 * dy + dx:5 * dy + dx + 1]
                if first:
                    nc.vector.tensor_scalar(lp, src, wtap,
                                            b_sb["lepe_b"][:, jt, :],
                                            op0=ALU.mult, op1=ALU.add)
                    first = False
                else:
                    nc.vector.scalar_tensor_tensor(lp, src, wtap, lp,
                                                   op0=ALU.mult, op1=ALU.add)

    # ---- Phase 2: attention, two 4-head supergroups ----
    for g in range(2):
        with tc.tile_pool(name=f"op{g}_{it}", bufs=1, space="PSUM") as op:
            o_t = [op.tile([128, 512], F32, tag=f"o_t{pr}", name=f"o_t{pr}")
                   for pr in range(2)]
            o_s = op.tile([128, 2, 64], F32, tag="o_s", name="o_s")
            with tc.tile_pool(name=f"sp{g}_{it}", bufs=2, space="PSUM") as sp, \
                 tc.tile_pool(name=f"stp{g}_{it}", bufs=1, space="PSUM") as stp, \
                 tc.tile_pool(name=f"mp{g}_{it}", bufs=5) as mp, \
                 tc.tile_pool(name=f"pp{g}_{it}", bufs=4) as pp:
                live = {}

                def emit_s(mt):
                    if g == 0 and mt < 6:
                        msk = msk_pre[mt]
                    else:
                        msk = mp.tile([128, 4, NQ], BF16, tag="msk",
                                      name="msk")
                        mio = io["maskT"]
                        src = bass.AP(
                            tensor=mio.tensor,
                            offset=mio.offset + (4 * g) * N * NQ
                            + 128 * mt * NQ,
                            ap=[[NQ, 128], [N * NQ, 4], [1, NQ]])
                        if "maskdma" not in ABLATE:
                            dma(msk[:], src)
                        else:
                            nc.vector.memset(msk[:], 1.0)
                    s_pair = [sp.tile([128, 2, 512], F32, tag="s", name="s")
                              for _ in range(2)]
                    stub = stp.tile([128, 4, 64], F32, tag="stub", name="stub")
                    for j in range(4) if "smm" not in ABLATE else []:
                        pr, ln_ = j // 2, j % 2
                        lhs = kr_bf[g][32 * j:32 * (j + 1),
                                       128 * mt:128 * (mt + 1)]
                        rq = qr_bf[g]
                        # inject mask via identity matmul: provides the
                        # full-row start of the PSUM group (a row-positioned
                        # start=True wedges the PE on this hardware)
                        nc.tensor.matmul(s_pair[pr][:, ln_, :], id_sb[:],
                                         msk[:, j, 0:512],
                                         start=True, stop=False)
                        nc.tensor.matmul(s_pair[pr][:, ln_, :], lhs,
                                         rq[32 * j:32 * (j + 1), 0:512],
                                         start=False, stop=True,
                                         tile_position=(32 * j, 0))
                        nc.tensor.matmul(stub[:, j, :], id_sb[:],
                                         msk[:, j, 512:576],
                                         start=True, stop=False)
                        nc.tensor.matmul(stub[:, j, :], lhs,
                                         rq[32 * j:32 * (j + 1), 512:576],
                                         start=False, stop=True,
                                         tile_position=(32 * j, 0))
                    if "smm" in ABLATE:
                        for pr in range(2):
                            nc.vector.memset(s_pair[pr][:], 0.01)
                        nc.vector.memset(stub[:], 0.01)
                    live[mt] = (msk, s_pair, stub)

                def emit_epa(mt):
                    # exp + mask-multiply + attn@v for tile mt; issued after
                    # s(mt+1) so the PE stream never stalls on the Act/DVE
                    # chain of the current tile.
                    msk, s_pair, stub = live.pop(mt)
                    p_sb = pp.tile([128, 4, NQ], BF16, tag="p_sb", name="p_sb")
                    if "exp" in ABLATE:
                        nc.vector.memset(p_sb[:], 0.001)
                    if "exp" not in ABLATE:
                        for pr in range(2):
                            nc.scalar.activation(p_sb[:, 2 * pr:2 * pr + 2, 0:512],
                                                 s_pair[pr][:], AF.Exp)
                        nc.scalar.activation(p_sb[:, :, 512:576], stub[:], AF.Exp)
                    for j in range(4) if "av" not in ABLATE else []:
                        pr, ln_ = j // 2, j % 2
                        h = 4 * g + j
                        lhsv = vn_bf[:, mt, h, :]
                        nc.tensor.matmul(o_t[pr][64 * ln_:64 * ln_ + 33, :],
                                         lhsv, p_sb[:, j, 0:512],
                                         start=(mt == 0), stop=(mt == MT - 1),
                                         tile_position=(0, 64 * ln_))
                        nc.tensor.matmul(o_s[64 * ln_:64 * ln_ + 33, pr, :],
                                         lhsv, p_sb[:, j, 512:576],
                                         start=(mt == 0), stop=(mt == MT - 1),
                                         tile_position=(0, 64 * ln_))

                if "noskew" in ABLATE:
                    for mt in range(MT):
                        emit_s(mt)
                        emit_epa(mt)
                else:
                    for mt in range(MT):
                        emit_s(mt)
                        if mt > 1:
                            emit_epa(mt - 2)
                    emit_epa(MT - 2)
                    emit_epa(MT - 1)
            if "av" in ABLATE:
                nc.vector.memset(oat_sb[g][:], 0.01)
                nc.vector.tensor_tensor(of_sb[g][:], oat_sb[g][:],
                                        lepe_sb[g][:], op=ALU.add)
                continue
            # evacuate lane-preserving, then DMA-remap rows:
            # head j=2*pr+ln at psum rows 64*ln..64*ln+32 of tile pr,
            # denominator at row 64*ln+32.
            for pr in range(2):
                nc.vector.tensor_copy(ofull[g][:, pr, 0:512], o_t[pr][:])
                nc.vector.tensor_copy(ofull[g][:, pr, 512:576], o_s[:, pr, :])
            for pr in range(2):
                dma(rden4[g][2 * pr:2 * pr + 2, :],
                    ofull[g][32:128:64, pr, :])
            for j in range(4):
                pr, ln_ = j // 2, j % 2
                dma(oat_sb[g][32 * j:32 * (j + 1), :],
                    ofull[g][64 * ln_:64 * ln_ + 32, pr, :])
            nc.vector.reciprocal(rden4[g][:], rden4[g][:])
            with tc.tile_pool(name=f"rb{g}_{it}", bufs=1, space="PSUM") as rbp:
                rb_m = rbp.tile([128, 512], F32, tag="rb_m", name="rb_m")
                rb_s = rbp.tile([128, 64], F32, tag="rb_s", name="rb_s")
                nc.tensor.matmul(rb_m[:], bi_sb[:], rden4[g][:, 0:512],
                                 start=True, stop=True)
                nc.tensor.matmul(rb_s[:], bi_sb[:], rden4[g][:, 512:576],
                                 start=True, stop=True)
                nc.vector.tensor_copy(rb_sb[:, 0:512], rb_m[:])
                nc.vector.tensor_copy(rb_sb[:, 512:576], rb_s[:])
            nc.vector.tensor_tensor(oat_sb[g][:], oat_sb[g][:], rb_sb[:],
                                    op=ALU.mult)
        nc.vector.tensor_tensor(of_sb[g][:], oat_sb[g][:], lepe_sb[g][:],
                                op=ALU.add)

    # ---- Phase 3: out-projection + residual + layernorm ----
    with tc.tile_pool(name=f"pwp_{it}", bufs=2, space="PSUM") as pwp:
        for jt in range(2):
            for off, wd in QCH:
                ps = pwp.tile([128, 512], F32, tag="pw", name="pw")
                for kk in range(2):
                    nc.tensor.matmul(ps[:, :wd],
                                     w_sb["wo"][:, kk, 128 * jt:128 * (jt + 1)],
                                     of_sb[kk][:, off:off + wd],
                                     start=(kk == 0), stop=(kk == 1))
                nc.vector.scalar_tensor_tensor(o2_sb[jt][:, off:off + wd],
                                               ps[:, :wd],
                                               b_sb["bo"][:, jt, :],
                                               xq_sb[jt][:, off:off + wd],
                                               op0=ALU.add, op1=ALU.add)
            nc.vector.tensor_tensor(sq_sb[jt][:], o2_sb[jt][:], o2_sb[jt][:],
                                    op=ALU.mult)
    with tc.tile_pool(name=f"stat_{it}", bufs=1, space="PSUM") as st:
        mu, ssq = {}, {}
        for off, wd in QCH:
            mu[off] = st.tile([1, wd], F32, tag=f"mu{off}", name="mu")
            ssq[off] = st.tile([1, wd], F32, tag=f"ssq{off}", name="ssq")
            for jt in range(2):
                nc.tensor.matmul(mu[off][:], o1_sb[:],
                                 o2_sb[jt][:, off:off + wd],
                                 start=(jt == 0), stop=(jt == 1))
                nc.tensor.matmul(ssq[off][:], o1_sb[:],
                                 sq_sb[jt][:, off:off + wd],
                                 start=(jt == 0), stop=(jt == 1))
        for off, wd in QCH:
            sl = slice(off, off + wd)
            nc.vector.tensor_scalar_mul(m1_sb[:, sl], mu[off][:], 1.0 / C)
            nc.vector.tensor_tensor(msq_sb[:, sl], m1_sb[:, sl],
                                    m1_sb[:, sl], op=ALU.mult)
            nc.vector.scalar_tensor_tensor(var_sb[:, sl], ssq[off][:],
                                           1.0 / C, msq_sb[:, sl],
                                           op0=ALU.mult, op1=ALU.subtract)
        nc.scalar.activation(rstd_sb[:], var_sb[:], AF.Sqrt, bias=eps_sb[:])
        nc.vector.reciprocal(rstd_sb[:], rstd_sb[:])
    with tc.tile_pool(name=f"bc_{it}", bufs=1, space="PSUM") as bc:
        mb = bc.tile([128, 512], F32, tag="mb", name="mb")
        mbs = bc.tile([128, 64], F32, tag="mbs", name="mbs")
        rbm = bc.tile([128, 512], F32, tag="rbm", name="rbm")
        rbs = bc.tile([128, 64], F32, tag="rbs", name="rbs")
        nc.tensor.matmul(mb[:], ok1_sb[:], m1_sb[:, 0:512],
                         start=True, stop=True)
        nc.tensor.matmul(mbs[:], ok1_sb[:], m1_sb[:, 512:576],
                         start=True, stop=True)
        nc.tensor.matmul(rbm[:], ok1_sb[:], rstd_sb[:, 0:512],
                         start=True, stop=True)
        nc.tensor.matmul(rbs[:], ok1_sb[:], rstd_sb[:, 512:576],
                         start=True, stop=True)
        mb_sb = sq_sb[0]  # scratch reuse
        rs_sb = rb_sb
        nc.vector.tensor_copy(mb_sb[:, 0:512], mb[:])
        nc.vector.tensor_copy(mb_sb[:, 512:576], mbs[:])
        nc.vector.tensor_copy(rs_sb[:, 0:512], rbm[:])
        nc.vector.tensor_copy(rs_sb[:, 512:576], rbs[:])
    for jt in range(2):
        t1 = oat_sb[jt]  # scratch reuse
        nc.vector.tensor_tensor(t1[:], o2_sb[jt][:], mb_sb[:],
                                op=ALU.subtract)
        nc.vector.tensor_tensor(t1[:], t1[:], rs_sb[:], op=ALU.mult)
        nc.vector.affine_then_add(outf_sb[jt][:], t1[:], o2_sb[jt][:],
                                  b_sb["ln_g"][:, jt, :],
                                  b_sb["ln_b"][:, jt, :])
        dma(io["outT"][128 * jt:128 * (jt + 1), :], outf_sb[jt][:])


def _host_inputs(x, sin, cos, mask, wq, bq, wk, bk, wv, bv,
                 lepe_w, lepe_b, wo, bo, ln_g, ln_b):
    bf = ml_dtypes.bfloat16
    maskT = np.ascontiguousarray(
        np.transpose(np.asarray(mask, np.float32), (0, 2, 1))).astype(bf)
    pm1 = np.tile(np.array([-1.0, 1.0], np.float32), HD // 2).reshape(HD, 1)
    sinT = np.ascontiguousarray(
        (np.asarray(sin, np.float32).reshape(N, HD).T * pm1).astype(bf))
    cosT = np.ascontiguousarray(
        np.asarray(cos, np.float32).reshape(N, HD).T.astype(bf))
    col = lambda a: np.asarray(a, np.float32).reshape(C, 1)
    common = dict(
        sinT=sinT, cosT=cosT,
        wq=np.asarray(wq, np.float32).astype(bf),
        wk=np.asarray(wk, np.float32).astype(bf),
        wv=np.asarray(wv, np.float32).astype(bf),
        wo=np.asarray(wo, np.float32).astype(bf),
        bq=col(bq), bk=col(bk), bv=col(bv), bo=col(bo),
        bv_row=np.asarray(bv, np.float32).reshape(1, C),
        lepe_w=np.asarray(lepe_w, np.float32).reshape(C, 25),
        lepe_b=col(lepe_b), ln_g=col(ln_g), ln_b=col(ln_b),
        ones_p=np.ones((128, 1), np.float32),
        ident_bf=np.eye(128, dtype=bf),
        ones_k1=np.ones((1, 128), np.float32),
        blockind=np.repeat(np.eye(4, dtype=np.float32), 32, axis=1
                           ).reshape(4, 128),
    )
    xs = np.asarray(x, np.float32).reshape(B, N, C)
    in_maps = []
    for c in range(NCORES):
        bb, qs = c // 4, c % 4
        xTb = np.ascontiguousarray(xs[bb].T)
        q0 = qs * NQ
        r0 = qs * ROWS_Q - 2
        halo = np.zeros((C, NHT), bf)
        vmask = np.zeros((1, NHT), np.float32)
        for r in range(HALO):
            ri = r0 + r
            if 0 <= ri < H:
                halo[:, r * W:(r + 1) * W] = xTb[:, ri * W:(ri + 1) * W].astype(bf)
                vmask[0, r * W:(r + 1) * W] = 1.0
        xq_f = np.ascontiguousarray(xTb[:, q0:q0 + NQ])
        m = dict(common)
        m.update(xT=xTb.astype(bf), xTq=xq_f, xTq_bf=xq_f.astype(bf),
                 xTh=halo, valid=vmask,
                 maskT=np.ascontiguousarray(maskT[:, :, q0:q0 + NQ]))
        in_maps.append(m)
    return in_maps


def _get_prog(iters=1):
    key = (iters, tuple(sorted(ABLATE)))
    if key not in _PROGS:
        _PROGS[key] = _build_program(iters)
    return _PROGS[key]


class _CachedRunner:
    """jax.jit callable built once per program; inputs device-put once per
    in_maps object, so repeated calls measure device execution only."""

    def __init__(self, nc, n_cores=NCORES):
        import jax
        from jax.sharding import Mesh, PartitionSpec
        from jax.experimental.shard_map import shard_map
        from concourse import bass2jax

        bass2jax.install_neuronx_cc_hook()
        self._jax = jax
        part = nc.partition_id_tensor.name if nc.partition_id_tensor else None
        in_names, out_names, out_avals = [], [], []
        for alloc in nc.m.functions[0].allocations:
            if not isinstance(alloc, mybir.MemoryLocationSet):
                continue
            name = alloc.memorylocations[0].name
            if alloc.kind == "ExternalInput":
                if name != part:
                    in_names.append(name)
            elif alloc.kind == "ExternalOutput":
                out_names.append(name)
                out_avals.append(jax.core.ShapedArray(
                    tuple(alloc.tensor_shape), mybir.dt.np(alloc.dtype)))
        all_in = list(in_names) + list(out_names)
        if part is not None:
            all_in.append(part)

        def _body(*args):
            operands = list(args)
            if part is not None:
                operands.append(bass2jax.partition_id_tensor())
            return tuple(bass2jax._bass_exec_p.bind(
                *operands,
                out_avals=tuple(out_avals),
                in_names=tuple(all_in),
                out_names=tuple(out_names),
                lowering_input_output_aliases=(),
                sim_require_finite=True,
                sim_require_nnan=True,
                nc=nc,
            ))

        devices = jax.devices()[:n_cores]
        mesh = Mesh(np.asarray(devices), ("core",))
        spec = PartitionSpec("core")
        self.fn = jax.jit(
            shard_map(_body, mesh=mesh,
                      in_specs=(spec,) * (len(in_names) + len(out_names)),
                      out_specs=(spec,) * len(out_names), check_rep=False),
            keep_unused=True)
        self.mesh, self.spec = mesh, spec
        self.in_names, self.out_names, self.out_avals = \
            in_names, out_names, out_avals
        self.n_cores = n_cores
        self._dev_cache = (None, None)

    def _device_inputs(self, in_maps):
        import jax
        from jax.sharding import NamedSharding
        key = id(in_maps)
        if self._dev_cache[0] == key:
            return self._dev_cache[1]
        sh = NamedSharding(self.mesh, self.spec)
        dev_in = [jax.device_put(np.concatenate(
            [np.asarray(in_maps[c][n]) for c in range(self.n_cores)], 0), sh)
            for n in self.in_names]
        dev_zero = [jax.device_put(np.zeros(
            (self.n_cores * a.shape[0], *a.shape[1:]), a.dtype), sh)
            for a in self.out_avals]
        self._dev_cache = (key, (dev_in, dev_zero))
        return dev_in, dev_zero

    def __call__(self, in_maps):
        dev_in, dev_zero = self._device_inputs(in_maps)
        outs = self.fn(*dev_in, *dev_zero)
        self._jax.block_until_ready(outs)
        return _LazyResults(outs, self.out_names, self.out_avals,
                            self.n_cores)


class _LazyResults:
    def __init__(self, outs, out_names, out_avals, n_cores):
        self._outs, self._names, self._avals, self._n = \
            outs, out_names, out_avals, n_cores

    @property
    def results(self):
        host = [np.asarray(o) for o in self._outs]
        return [
            {n: host[i].reshape(self._n, *self._avals[i].shape)[c]
             for i, n in enumerate(self._names)}
            for c in range(self._n)
        ]


def _get_runner(iters=1):
    key = (iters, tuple(sorted(ABLATE)))
    if key not in _RUNNERS:
        _RUNNERS[key] = _CachedRunner(_get_prog(iters))
    return _RUNNERS[key]


def run_with_iters(in_maps, iters=1):
    return _get_runner(iters)(in_maps)


def host_inputs(**inputs):
    return _host_inputs(**inputs)


def _gather(results):
    out = np.empty((B, N, C), np.float32)
    for c in range(NCORES):
        bb, qs = c // 4, c % 4
        out[bb, qs * NQ:(qs + 1) * NQ, :] = results[c]["outT"].T
    return out.reshape(B, H, W, C)


def kernel(**inputs):
    in_maps = _host_inputs(**inputs)
    res = bass_utils.run_bass_kernel_spmd(_get_prog(1), in_maps,
                                          core_ids=list(range(NCORES)))
    return _gather(res.results)
